# revision 19
# baseline (speedup 1.0000x reference)
"""DTGNN Trainium2 Bass kernel (v2, latency-optimized).

Single-core algorithm (graph tiny: N=8, E=16), replicated across 8 NeuronCores
via SPMD; core 0's output returned. Optimizations vs v1:
  - bf16 matmul inputs for all wide matmuls (4x PE throughput); fp32 kept on
    the attention/softmax (alpha) path.
  - GAT attention reductions (h*a_s).sum folded into the X@W matmul as extra
    host-precomputed columns (W @ a_s); edge-attn term ae = ea @ (We@a_e).
  - CNN_2 tail (conv2+flatten+linear1+linear2) collapsed into 4 accumulating
    [33,10]x[33,64] matmuls with host-folded weights; conv1 bias commutes
    with maxpool and folds into the same constant row.
  - Biases folded as ones-row contraction tricks (no separate DVE adds).
  - Edge-MLP mean-pool (ef/er) computed as sums of the hidden layer pushed
    through (w2/8 | b2) on the PE.
  - 3 input DMAs (was 6), packed mixed-dtype via bf16-pair bitcasting.
"""
import numpy as np
import ml_dtypes
from contextlib import ExitStack

import concourse.bacc as bacc
import concourse.bass as bass
import concourse.tile as tile
import concourse.mybir as mybir
from concourse.bass_utils import run_bass_kernel_spmd

F32 = mybir.dt.float32
BF16 = mybir.dt.bfloat16
I32 = mybir.dt.int32
ALU = mybir.AluOpType
ACT = mybir.ActivationFunctionType
AXL = mybir.AxisListType


def _mkoff(lst):
    d, o = {}, 0
    for name, w in lst:
        d[name] = o
        o += w
    d["_W"] = o
    return d


# tA [33, *] f32 — constants + CNN1 + one-hot bits (lands first)
_LA = [("W1s", 8), ("TPAD3", 20), ("w2T", 3), ("mask16", 8), ("M24x", 24),
       ("iota_row24", 8), ("iota8", 1), ("ipack", 50), ("c1b1", 1),
       ("c1b2", 1), ("e17c", 1), ("ident16f", 16), ("ident8b", 4)]
# tB [128, *] f32 — GAT1 h weights + alpha1 + MLP (lands second)
_LB = [("XTb", 16), ("XTf", 32), ("G1b", 512), ("G1f", 32), ("W16t", 128),
       ("W16as", 8), ("eaT24", 24), ("Wae", 4), ("eaTb", 8), ("mlpw1b", 32),
       ("mlpb1", 1), ("wv2", 1)]
# tC [128, *] f32 — GAT2 + deconv + CNN2 weights (lands third)
_LC = [("G2b", 66), ("mlpw2e", 64), ("D1b", 320), ("D2b", 320),
       ("c2w1T", 48), ("A33", 20), ("g1brow", 128), ("g2brow", 64),
       ("cidx68", 4), ("pcol68", 1)]

_oA, _oB, _oC = _mkoff(_LA), _mkoff(_LB), _mkoff(_LC)
_WA = ((_oA["_W"] + 127) // 128) * 128
_WB = _oB["_W"]
_WC = _oC["_W"]


def _build_nc(stage=99):
    nc = bacc.Bacc("TRN2", target_bir_lowering=False)

    mA = nc.dram_tensor("mA", [33, _WA], F32, kind="ExternalInput")
    mB = nc.dram_tensor("mB", [128, _WB], F32, kind="ExternalInput")
    mC = nc.dram_tensor("mC", [128, _WC], F32, kind="ExternalInput")
    out = nc.dram_tensor("out", [10, 64], F32, kind="ExternalOutput")

    with tile.TileContext(nc) as tc, ExitStack() as ctx:
        sb = ctx.enter_context(tc.tile_pool(name="sb", bufs=1))
        ps = ctx.enter_context(tc.tile_pool(name="ps", bufs=8, space="PSUM"))
        ctx.enter_context(nc.allow_low_precision(reason="bf16 kernel"))

        def _go():
            # --------------------------------------------- input DMAs
            tA = sb.tile([33, _WA], F32)
            nc.sync.dma_start(tA[:], mA[:])
            tB = sb.tile([128, _WB], F32)
            nc.sync.dma_start(tB[:], mB[:])
            tC = sb.tile([128, _WC], F32)
            nc.sync.dma_start(tC[:], mC[:])

            def A(name, w, rows, r0=0):
                return tA[r0:r0 + rows, _oA[name]:_oA[name] + w]

            def Bv(name, w, rows, r0=0):
                return tB[r0:r0 + rows, _oB[name]:_oB[name] + w]

            def C(name, w, rows, r0=0):
                return tC[r0:r0 + rows, _oC[name]:_oC[name] + w]

            W1s = A("W1s", 8, 24)
            TPAD3 = A("TPAD3", 20, 24).rearrange("p (b t) -> p b t", b=2)
            w2T = A("w2T", 3, 10)
            mask16 = A("mask16", 8, 16)
            M24x = A("M24x", 24, 16)
            iota_row24 = A("iota_row24", 8, 24)
            iota8 = A("iota8", 1, 8)
            c1b1 = A("c1b1", 1, 8)
            c1b2 = A("c1b2", 1, 1)
            e17c = A("e17c", 1, 1)
            ident16f = A("ident16f", 16, 16)
            ident8b = A("ident8b", 4, 8).bitcast(BF16)

            XTb = Bv("XTb", 16, 128).bitcast(BF16).rearrange("p (j n) -> p j n", j=4)
            XTf = Bv("XTf", 32, 128).rearrange("p (j n) -> p j n", j=4)
            G1b = Bv("G1b", 512, 128).bitcast(BF16).rearrange("p (j n) -> p j n", j=4)
            G1f = Bv("G1f", 32, 128).rearrange("p (j n) -> p j n", j=4)
            W16t = Bv("W16t", 128, 16).bitcast(BF16)
            W16as = Bv("W16as", 8, 16)
            eaT24 = Bv("eaT24", 24, 128)
            Wae = Bv("Wae", 4, 128)
            eaTb = Bv("eaTb", 8, 128).bitcast(BF16)
            mlpw1b = Bv("mlpw1b", 32, 128).bitcast(BF16)
            mlpb1 = Bv("mlpb1", 1, 64)
            wv2 = Bv("wv2", 1, 64)

            G2b = C("G2b", 66, 128).bitcast(BF16).rearrange("p (j n) -> p j n", j=2)
            mlpw2e = C("mlpw2e", 64, 65)
            D1b = C("D1b", 320, 128).bitcast(BF16)
            D2b = C("D2b", 320, 68).bitcast(BF16)
            c2w1T = C("c2w1T", 48, 4).bitcast(BF16).rearrange("p (k n) -> p k n", k=3)
            A33 = C("A33", 20, 33).bitcast(BF16).rearrange("p (j n) -> p j n", j=4)
            g1brow = C("g1brow", 128, 1).bitcast(BF16)
            cidx68 = C("cidx68", 4, 68)
            pcol68 = C("pcol68", 1, 68)
            g2brow = C("g2brow", 64, 1)

            # --------------------------------------------- early memsets
            zp = sb.tile([10, 2, 10], F32)
            nc.vector.memset(zp[:], 0.0)
            warm = sb.tile([1, 1], F32)
            nc.vector.memset(warm[:], 0.0)
            warm2 = sb.tile([1, 1], F32)
            nc.scalar.activation(warm2[:], warm[:], ACT.Exp)
            sel = sb.tile([128, 4], BF16)
            nc.vector.memset(sel[:], 0.0)
            sel2 = sb.tile([68, 4], BF16)
            Pdst24b = sb.tile([24, 8], BF16)
            y1c33 = sb.tile([33, 64, 4], BF16)
            nc.vector.memset(y1c33[32:33, :, :], 1.0)
            s2 = sb.tile([65, 2], F32)
            nc.vector.memset(s2[64:65, :], 1.0)
            ones1x8 = sb.tile([1, 8], F32)
            nc.vector.memset(ones1x8[:], 1.0)
            ones8bb = sb.tile([1, 8], BF16)
            nc.vector.memset(ones8bb[:], 1.0)
            ones24f = sb.tile([1, 24], F32)
            nc.vector.memset(ones24f[:], 1.0)
            ones8b = sb.tile([8, 1], F32)
            nc.vector.memset(ones8b[:], 0.125)

            # --------------------------------------------- one-hot matrices
            ti = A("ipack", 50, 24).bitcast(I32)
            tif = sb.tile([24, 50], F32)
            nc.vector.tensor_copy(tif[:], ti)
            idx_f = tif[0:8, 0:48].rearrange("p (c e) -> p c e", c=2)
            dcol_f = tif[:, 48:49]

            PsrcTf = sb.tile([8, 24], F32)
            nc.vector.tensor_scalar(PsrcTf[:], idx_f[:, 0, :], iota8, None,
                                    ALU.is_equal)
            PsrcTb = sb.tile([8, 24], BF16)
            nc.vector.tensor_scalar(PsrcTb[:], idx_f[:, 0, :], iota8, None,
                                    ALU.is_equal)
            PdstTf = sb.tile([8, 24], F32)
            nc.vector.tensor_scalar(PdstTf[:], idx_f[:, 1, :], iota8, None,
                                    ALU.is_equal)
            Pdst24f = sb.tile([24, 8], F32)
            nc.vector.tensor_scalar(Pdst24f[:], iota_row24, dcol_f, None,
                                    ALU.is_equal)
            nc.vector.tensor_scalar(Pdst24b[:], iota_row24, dcol_f, None,
                                    ALU.is_equal)

            # --------------------------------------------- CNN_1
            ps_y1 = ps.tile([8, 2, 10], F32, tag="ps")
            nc.tensor.matmul(ps_y1[:], W1s, TPAD3, start=True, stop=True)
            y1 = sb.tile([8, 2, 10], F32)
            nc.vector.tensor_scalar(y1[:], ps_y1[:], c1b1, 0.0, ALU.add, ALU.max)

            ps_za = ps.tile([10, 8], F32, tag="ps")
            nc.tensor.transpose(ps_za[:], y1[:, 0, :], ident16f[0:8, 0:8])
            ps_zb = ps.tile([10, 8], F32, tag="ps")
            nc.tensor.transpose(ps_zb[:], y1[:, 1, :], ident16f[0:8, 0:8])
            nc.vector.tensor_copy(zp[:, 0, 1:9], ps_za[:])
            nc.scalar.copy(zp[:, 1, 1:9], ps_zb[:])

            ps_y2 = ps.tile([1, 16], F32, tag="ps")
            for k in range(3):
                nc.tensor.matmul(ps_y2[:], w2T[:, k:k + 1], zp[:, :, k:k + 8],
                                 start=(k == 0), stop=(k == 2))
            xr = sb.tile([1, 16], F32)
            nc.vector.tensor_scalar(xr[:].rearrange("p (c n) -> p n c", c=2),
                                    ps_y2[:].rearrange("p (n c) -> p n c", c=2),
                                    c1b2, 0.0, ALU.add, ALU.max)
            ps_xrT = ps.tile([16, 1], F32, tag="ps")
            nc.tensor.transpose(ps_xrT[:], xr[:], ident16f[0:1, 0:1])
            x16f = sb.tile([16, 8], F32)
            nc.vector.tensor_tensor(x16f[:], ps_xrT[:].broadcast_to([16, 8]),
                                    mask16, ALU.mult)
            x16b = sb.tile([16, 8], BF16)
            nc.vector.tensor_tensor(x16b[:], ps_xrT[:].broadcast_to([16, 8]),
                                    mask16, ALU.mult)
            if stage == 1:
                o10 = sb.tile([10, 64], F32)
                nc.vector.memset(o10[:], 0.0)
                nc.vector.tensor_copy(o10[0:8, 0:20],
                                      y1[:].rearrange("p b t -> p (b t)"))
                nc.sync.dma_start(out[:], o10[:])
                return

            # --------------------------------------------- GAT 1
            ps_hf = ps.tile([8, 8], F32, tag="ps")
            for j in range(4):
                nc.tensor.matmul(ps_hf[:], XTf[:, j, :], G1f[:, j, :],
                                 start=(j == 0), stop=False)
            nc.tensor.matmul(ps_hf[:], x16f[:], W16as, start=False, stop=True)
            ps_h = ps.tile([8, 256], F32, tag="ps")
            for j in range(4):
                nc.tensor.matmul(ps_h[:], XTb[:, j, :], G1b[:, j, :],
                                 start=(j == 0), stop=False)
            nc.tensor.matmul(ps_h[:], x16b[:], W16t, start=False, stop=True)

            asad = sb.tile([8, 8], F32)
            nc.vector.tensor_copy(asad[:], ps_hf[:])
            h_sb = sb.tile([8, 256], BF16)
            nc.vector.tensor_copy(h_sb[:], ps_h[:])

            if stage == 21:
                o10 = sb.tile([10, 64], F32)
                nc.vector.memset(o10[:], 0.0)
                nc.vector.tensor_copy(o10[0:8, 0:56], ps_h[:, 0:56])
                nc.vector.tensor_copy(o10[0:8, 56:64], asad[:])
                nc.sync.dma_start(out[:], o10[:])
                return

            ps_al = ps.tile([24, 4], F32, tag="ps")
            nc.tensor.matmul(ps_al[:], eaT24, Wae, start=True, stop=False)
            nc.tensor.matmul(ps_al[:], PsrcTf[:], asad[:, 0:4], start=False,
                             stop=False)
            nc.tensor.matmul(ps_al[:], PdstTf[:], asad[:, 4:8], start=False,
                             stop=True)
            al1 = sb.tile([24, 4], F32)
            nc.vector.tensor_copy(al1[:], ps_al[:])
            ps_sg = ps.tile([24, 256], F32, tag="ps")
            nc.tensor.matmul(ps_sg[:], PsrcTb[:], h_sb[:], start=True, stop=True)
            lr1 = sb.tile([24, 4], F32)
            nc.vector.scalar_tensor_tensor(lr1[:], al1[:], 0.2, al1[:],
                                           ALU.mult, ALU.max)
            ex24 = sb.tile([24, 4], F32)
            nc.scalar.activation(ex24[:], lr1[:], ACT.Exp)
            ps_den = ps.tile([8, 4], F32, tag="ps")
            nc.tensor.matmul(ps_den[:], Pdst24f[:], ex24[:], start=True, stop=True)
            rden = sb.tile([8, 4], F32)
            nc.vector.reciprocal(rden[:], ps_den[:])
            ps_rde = ps.tile([24, 4], F32, tag="ps")
            nc.tensor.matmul(ps_rde[:], PdstTf[:], rden[:], start=True, stop=True)
            wexp = sb.tile([24, 4], F32)
            nc.vector.tensor_tensor(wexp[:], ex24[:], ps_rde[:], ALU.mult)

            if stage == 22:
                o10 = sb.tile([10, 64], F32)
                nc.vector.memset(o10[:], 0.0)
                nc.vector.tensor_copy(o10[0:8, 0:4], rden[:])
                nc.vector.tensor_copy(o10[0:8, 8:12], ps_den[:])
                nc.sync.dma_start(out[:], o10[:])
                return
            wh24 = sb.tile([24, 256], BF16)
            nc.vector.tensor_tensor(
                wh24[:].rearrange("p (h c) -> p h c", h=4),
                ps_sg[:].rearrange("p (h c) -> p h c", h=4),
                wexp[:].broadcast_to([24, 4, 64]), ALU.mult)
            ps_x1 = ps.tile([8, 256], F32, tag="ps")
            nc.tensor.matmul(ps_x1[:], ones8bb[:], g1brow, start=True, stop=False)
            nc.tensor.matmul(ps_x1[:], Pdst24b[:], wh24[:], start=False, stop=True)
            x1 = sb.tile([8, 256], BF16)
            nc.vector.tensor_scalar(x1[:], ps_x1[:], 0.0, None, ALU.max)
            if stage == 2:
                o10 = sb.tile([10, 64], F32)
                nc.vector.memset(o10[:], 0.0)
                nc.vector.tensor_copy(o10[0:8, 0:64], x1[:, 0:64])
                nc.sync.dma_start(out[:], o10[:])
                return

            nc.vector.tensor_scalar(sel2[:], cidx68, pcol68, None, ALU.is_equal)
            # --------------------------------------------- edge MLP (early)
            ps_m1 = ps.tile([64, 16], F32, tag="ps")
            nc.tensor.matmul(ps_m1[:], mlpw1b, eaTb, start=True, stop=True)
            r1T = sb.tile([64, 16], F32)
            nc.vector.tensor_scalar(r1T[:], ps_m1[:], mlpb1, 0.0, ALU.add,
                                    ALU.max)
            r1v = r1T[:].rearrange("p (e two) -> p two e", two=2)
            nc.vector.tensor_reduce(s2[0:64, 0:1], r1v[:, 0, :], axis=AXL.X,
                                    op=ALU.add)
            nc.vector.tensor_reduce(s2[0:64, 1:2], r1v[:, 1, :], axis=AXL.X,
                                    op=ALU.add)
            # ef into partitions 64:128 (for sel), er into 0:64 (for sel2)
            ps_ee = ps.tile([128, 2], F32, tag="ps")
            nc.tensor.matmul(ps_ee[64:128, 0:1], mlpw2e, s2[:, 0:1], start=True,
                             stop=True)
            nc.tensor.matmul(ps_ee[0:64, 1:2], mlpw2e, s2[:, 1:2], start=True,
                             stop=True)
            nc.vector.tensor_copy(sel[64:128, 2:3], ps_ee[64:128, 0:1])
            nc.vector.tensor_copy(sel2[0:64, 3:4], ps_ee[0:64, 1:2])

            ps_e16 = ps.tile([16, 1], F32, tag="ps")
            nc.tensor.matmul(ps_e16[:], r1T[:], wv2, start=True, stop=True)
            e16sb = sb.tile([16, 1], F32)
            nc.vector.tensor_copy(e16sb[:], ps_e16[:])
            b8sb = sb.tile([8, 64], F32)
            ps_b8 = ps.tile([8, 64], F32, tag="ps")
            nc.tensor.matmul(ps_b8[:], ones1x8[:], g2brow, start=True, stop=True)
            nc.vector.tensor_copy(b8sb[:], ps_b8[:])

            # --------------------------------------------- GAT 2
            ps_tr1 = ps.tile([128, 8], BF16, tag="ps")
            nc.tensor.transpose(ps_tr1[:], x1[:, 0:128], ident8b)
            ps_tr2 = ps.tile([128, 8], BF16, tag="ps")
            nc.tensor.transpose(ps_tr2[:], x1[:, 128:256], ident8b)
            x1T = sb.tile([128, 2, 8], BF16)
            nc.vector.tensor_copy(x1T[:, 0, :], ps_tr1[:])
            nc.vector.tensor_copy(x1T[:, 1, :], ps_tr2[:])

            ps_h2 = ps.tile([8, 66], F32, tag="ps")
            for j in range(2):
                nc.tensor.matmul(ps_h2[:], x1T[:, j, :], G2b[:, j, :],
                                 start=(j == 0), stop=(j == 1))
            hs2 = sb.tile([8, 66], F32)
            nc.vector.tensor_copy(hs2[:], ps_h2[:])

            ps_al2 = ps.tile([24, 1], F32, tag="ps")
            nc.tensor.matmul(ps_al2[:], ones24f[:], e17c, start=True, stop=False)
            nc.tensor.matmul(ps_al2[:], M24x, e16sb[:], start=False, stop=False)
            nc.tensor.matmul(ps_al2[:], PsrcTf[:], hs2[:, 64:65], start=False,
                             stop=False)
            nc.tensor.matmul(ps_al2[:], PdstTf[:], hs2[:, 65:66], start=False,
                             stop=True)
            al2 = sb.tile([24, 1], F32)
            nc.vector.tensor_copy(al2[:], ps_al2[:])
            lr2 = sb.tile([24, 1], F32)
            nc.vector.scalar_tensor_tensor(lr2[:], al2[:], 0.2, al2[:],
                                           ALU.mult, ALU.max)
            ex2 = sb.tile([24, 1], F32)
            nc.scalar.activation(ex2[:], lr2[:], ACT.Exp)
            ps_sg2 = ps.tile([24, 64], F32, tag="ps")
            nc.tensor.matmul(ps_sg2[:], PsrcTf[:], hs2[:, 0:64], start=True,
                             stop=True)
            ps_den2 = ps.tile([8, 1], F32, tag="ps")
            nc.tensor.matmul(ps_den2[:], Pdst24f[:], ex2[:], start=True, stop=True)
            rden2 = sb.tile([8, 1], F32)
            nc.vector.reciprocal(rden2[:], ps_den2[:])
            wh2 = sb.tile([24, 64], BF16)
            nc.vector.tensor_scalar(wh2[:], ps_sg2[:], ex2[:], None, ALU.mult)
            ps_x2u = ps.tile([8, 64], F32, tag="ps")
            nc.tensor.matmul(ps_x2u[:], Pdst24b[:], wh2[:], start=True,
                             stop=True)
            x2t = sb.tile([8, 64], F32)
            nc.vector.scalar_tensor_tensor(x2t[:], ps_x2u[:], rden2[:], b8sb[:],
                                           ALU.mult, ALU.add)
            x2 = sb.tile([8, 64], F32)
            nc.vector.tensor_scalar(x2[:], x2t[:], 0.0, None, ALU.max)
            if stage == 3:
                o10 = sb.tile([10, 64], F32)
                nc.vector.memset(o10[:], 0.0)
                nc.vector.tensor_copy(o10[0:8, 0:64], x2[:])
                nc.sync.dma_start(out[:], o10[:])
                return

            # --------------------------------------------- deconv selector
            ps_xm = ps.tile([64, 1], F32, tag="ps")
            nc.tensor.matmul(ps_xm[:], x2[:], ones8b[:], start=True, stop=True)
            nc.vector.tensor_copy(sel[0:64, 0:1], ps_xm[:])

            ps_cTb = ps.tile([4, 320], F32, tag="ps")
            nc.tensor.matmul(ps_cTb[:], sel[:], D1b[:, 320:640], start=True,
                             stop=False)
            nc.tensor.matmul(ps_cTb[:], sel2[:], D2b[:, 320:640], start=False,
                             stop=True)
            ps_cTa = ps.tile([4, 320], F32, tag="ps")
            nc.tensor.matmul(ps_cTa[:], sel[:], D1b[:, 0:320], start=True,
                             stop=False)
            nc.tensor.matmul(ps_cTa[:], sel2[:], D2b[:, 0:320], start=False,
                             stop=True)
            cT = sb.tile([4, 640], BF16)
            nc.scalar.copy(cT[:, 320:640], ps_cTb[:])
            nc.vector.tensor_copy(cT[:, 0:320], ps_cTa[:])
            if stage == 4:
                o10 = sb.tile([10, 64], F32)
                nc.vector.memset(o10[:], 0.0)
                nc.vector.tensor_copy(o10[0:4, 0:40], cT[:, 0:40])
                nc.sync.dma_start(out[:], o10[:])
                return

            # --------------------------------------------- CNN_2
            cTv = cT[:].rearrange("p (b l) -> p b l", b=64)
            ps_c1b = ps.tile([32, 32, 8], F32, tag="ps")
            for k in range(3):
                nc.tensor.matmul(ps_c1b[:], c2w1T[:, k, :],
                                 cTv[:, 32:64, k:k + 8],
                                 start=(k == 0), stop=(k == 2))
            ps_c1a = ps.tile([32, 32, 8], F32, tag="ps")
            for k in range(3):
                nc.tensor.matmul(ps_c1a[:], c2w1T[:, k, :],
                                 cTv[:, 0:32, k:k + 8],
                                 start=(k == 0), stop=(k == 2))
            nc.vector.tensor_reduce(
                y1c33[0:32, 32:64, :],
                ps_c1b[:].rearrange("p b (l two) -> p b l two", two=2),
                axis=AXL.X, op=ALU.max)
            nc.vector.tensor_reduce(
                y1c33[0:32, 0:32, :],
                ps_c1a[:].rearrange("p b (l two) -> p b l two", two=2),
                axis=AXL.X, op=ALU.max)

            ps_out = ps.tile([10, 64], F32, tag="ps")
            for j in range(4):
                nc.tensor.matmul(ps_out[:], A33[:, j, :], y1c33[:, :, j],
                                 start=(j == 0), stop=(j == 3))
            o10 = sb.tile([10, 64], F32)
            nc.vector.tensor_scalar(o10[:], ps_out[:], 0.0, None, ALU.max)
            nc.sync.dma_start(out[:], o10[:])

        _go()
    nc.finalize()
    return nc


_NC = None


def _get_nc():
    global _NC
    if _NC is None:
        _NC = _build_nc()
    return _NC


def _bfpack(a):
    """[r, c] float array -> [r, c/2] f32 whose bits hold bf16 pairs."""
    a = np.ascontiguousarray(np.asarray(a, dtype=np.float32))
    r, c = a.shape
    assert c % 2 == 0, c
    u = a.astype(ml_dtypes.bfloat16).view(np.uint16).reshape(r, c // 2, 2)
    packed = u[:, :, 0].astype(np.uint32) | (u[:, :, 1].astype(np.uint32) << 16)
    return packed.view(np.float32)


def _pack_inputs(x_feat, x_feat_tmp, edge_attr, c1w1, c1b1, c1w2, c1b2,
                 g1_lin, g1_as, g1_ad, g1_le, g1_ae, g1_b,
                 g2_lin, g2_as, g2_ad, g2_le, g2_ae, g2_b,
                 mlp_w1, mlp_b1, mlp_w2, mlp_b2,
                 d1w, d1b, d2w, d2b, d3w, d3b,
                 c2w1, c2b1, c2w2, c2b2, c2l1w, c2l1b, c2l2w, c2l2b,
                 edge_index):
    f = np.float32
    x_feat = np.asarray(x_feat, f)
    x_feat_tmp = np.asarray(x_feat_tmp, f)
    edge_attr = np.asarray(edge_attr, f)

    def fill(shape, off, blocks):
        arr = np.zeros(shape, dtype=f)
        for name, a in blocks.items():
            a = np.asarray(a, dtype=f)
            arr[0:a.shape[0], off[name]:off[name] + a.shape[1]] = a
        return arr

    # ---- tA ----
    tpad = np.zeros((8, 2, 12), dtype=f)
    for i in range(8):
        r = 1 if i % 2 == 0 else 5
        for b in range(2):
            tpad[i, b, 1:11] = x_feat_tmp[r, b * 4 + i // 2]
    T3 = np.zeros((24, 20), dtype=f)
    for k in range(3):
        for c in range(8):
            T3[k * 8 + c] = tpad[c, :, k:k + 10].reshape(20)
    W1s = np.asarray(c1w1, f).transpose(2, 1, 0).reshape(24, 8)

    mask16 = np.zeros((16, 8), dtype=f)
    for c in range(2):
        for n in range(8):
            mask16[c * 8 + n, n] = 1.0

    M24x = np.zeros((16, 24), dtype=f)
    M24x[0:16, 0:16] = np.eye(16, dtype=f)
    M24x[0:16, 16:24] = 1.0 / 16.0

    ve2 = np.asarray(g2_le, f) @ np.asarray(g2_ae, f).reshape(64)  # [64]
    wv2 = (np.asarray(mlp_w2, f) @ ve2).reshape(64, 1)
    e17c = float(np.asarray(mlp_b2, f) @ ve2)

    ipack = np.zeros((24, 50), dtype=np.int32)
    blk = np.zeros((8, 2, 24), dtype=np.int32)
    blk[:, :, 0:16] = np.asarray(edge_index, np.int32)[None, :, :]
    blk[:, :, 16:24] = np.arange(8, dtype=np.int32)[None, None, :]
    ipack[0:8, 0:48] = blk.reshape(8, 48)
    ipack[0:16, 48] = np.asarray(edge_index, np.int32)[1]
    ipack[16:24, 48] = np.arange(8, dtype=np.int32)

    tAm = fill((33, _WA), _oA, {
        "W1s": W1s, "TPAD3": T3,
        "w2T": np.asarray(c1w2, f).transpose(1, 2, 0).reshape(10, 3),
        "mask16": mask16, "M24x": M24x,
        "iota_row24": np.broadcast_to(np.arange(8, dtype=f), (24, 8)),
        "iota8": np.arange(8, dtype=f).reshape(8, 1),
        "ipack": ipack.view(np.float32),
        "c1b1": np.asarray(c1b1, f).reshape(8, 1),
        "c1b2": np.asarray(c1b2, f).reshape(1, 1),
        "ident16f": np.eye(16, dtype=f),
        "ident8b": _bfpack(np.eye(8, dtype=f)),
    })
    tAm[0, _oA["e17c"]] = e17c

    # ---- tB ----
    xfT = np.zeros((512, 8), dtype=f)
    xfT[0:510] = x_feat.T
    XT4 = xfT.reshape(4, 128, 8).transpose(1, 0, 2).reshape(128, 32)
    G1 = np.asarray(g1_lin, f)                       # [512, 256]
    Was = np.zeros((512, 4), dtype=f)
    Wad = np.zeros((512, 4), dtype=f)
    a_s = np.asarray(g1_as, f)
    a_d = np.asarray(g1_ad, f)
    for h in range(4):
        Was[:, h] = G1[:, h * 64:(h + 1) * 64] @ a_s[h]
        Wad[:, h] = G1[:, h * 64:(h + 1) * 64] @ a_d[h]
    G1f8 = np.concatenate([Was, Wad], 1)             # [512, 8]
    Wae = (np.asarray(g1_le, f).reshape(128, 4, 64) *
           np.asarray(g1_ae, f)[None]).sum(-1)       # [128, 4]
    ea_mean = edge_attr.mean(0)
    ea24 = np.concatenate([edge_attr, np.broadcast_to(ea_mean, (8, 128))], 0)

    tBm = fill((128, _WB), _oB, {
        "XTb": _bfpack(XT4),
        "XTf": XT4,
        "G1b": _bfpack(
            G1.reshape(4, 128, 256).transpose(1, 0, 2).reshape(128, 1024)),
        "G1f": G1f8.reshape(4, 128, 8).transpose(1, 0, 2).reshape(128, 32),
        "W16t": _bfpack(np.repeat(G1[510:512], 8, axis=0)),       # [16, 256]
        "W16as": np.repeat(G1f8[510:512], 8, axis=0),             # [16, 8]
        "eaT24": ea24.T,
        "Wae": Wae,
        "eaTb": _bfpack(edge_attr.T),
        "mlpw1b": _bfpack(np.asarray(mlp_w1, f)),
        "mlpb1": np.asarray(mlp_b1, f).reshape(64, 1),
        "wv2": wv2,
    })

    # ---- tC ----
    G2 = np.asarray(g2_lin, f)                        # [256, 64]
    was2 = (G2 @ np.asarray(g2_as, f).reshape(64)).reshape(256, 1)
    wad2 = (G2 @ np.asarray(g2_ad, f).reshape(64)).reshape(256, 1)
    G2e = np.concatenate([G2, was2, wad2], 1)         # [256, 66]
    G2e4 = G2e.reshape(2, 128, 66).transpose(1, 0, 2).reshape(128, 132)

    mlpw2e = np.zeros((65, 64), dtype=f)
    mlpw2e[0:64] = np.asarray(mlp_w2, f) * 0.125
    mlpw2e[64] = np.asarray(mlp_b2, f)

    D1 = np.concatenate([np.asarray(d1w, f).reshape(64, 640),
                         np.asarray(d2w, f).reshape(64, 640)], 0)
    b4 = np.zeros((4, 640), dtype=f)
    b4[0] = np.repeat(np.asarray(d1b, f), 10)
    b4[1] = x_feat_tmp.reshape(640)
    b4[2] = np.repeat(np.asarray(d2b, f), 10)
    b4[3] = np.repeat(np.asarray(d3b, f), 10)
    D2 = np.concatenate([np.asarray(d3w, f).reshape(64, 640), b4], 0)  # [68,640]

    # CNN2 fold: Wf [128,10] over (c2, l2); A[(c1,j), t]; const row.
    Wf = np.asarray(c2l1w, f) @ np.asarray(c2l2w, f)            # [128, 10]
    bfold = np.asarray(c2l1b, f) @ np.asarray(c2l2w, f) + np.asarray(c2l2b, f)
    WfR = Wf.reshape(64, 2, 10)                                 # [c2, l2, t]
    w2c = np.asarray(c2w2, f)                                   # [64, 32, 3]
    Afold = np.zeros((32, 4, 10), dtype=f)                      # [c1, j, t]
    for j in range(4):
        for l2 in range(2):
            k = j - l2
            if 0 <= k <= 2:
                Afold[:, j, :] += np.einsum("co,ct->ot", w2c[:, :, k],
                                            WfR[:, l2, :])
    const = (np.einsum("c,clt->t",
                       np.asarray(c2b2, f), WfR) + bfold +
             np.einsum("c,cjt->t", np.asarray(c2b1, f), Afold))
    A33m = np.zeros((33, 40), dtype=f)
    A33m[0:32] = Afold.reshape(32, 40)
    A33m[32, 0:10] = const
    tCm = fill((128, _WC), _oC, {
        "G2b": _bfpack(G2e4),
        "mlpw2e": mlpw2e,
        "D1b": _bfpack(D1),
        "D2b": _bfpack(D2),
        "c2w1T": _bfpack(np.asarray(c2w1, f).transpose(1, 2, 0).reshape(4, 96)),
        "A33": _bfpack(A33m),
        "g2brow": np.asarray(g2_b, f).reshape(1, 64),
        "g1brow": _bfpack(np.asarray(g1_b, f).reshape(1, 256)),
        "cidx68": np.broadcast_to(np.arange(4, dtype=f), (68, 4)),
        "pcol68": (np.arange(68, dtype=f) - 64.0).reshape(68, 1),
    })
    return tAm, tBm, tCm


def _make_ins(inputs):
    tAm, tBm, tCm = _pack_inputs(**inputs)
    return {"mA": tAm, "mB": tBm, "mC": tCm}


def kernel(**inputs):
    inputs = {k: np.ascontiguousarray(v) for k, v in inputs.items()}
    ins = _make_ins(inputs)
    nc = _get_nc()
    res = run_bass_kernel_spmd(nc, [ins] * 8, core_ids=list(range(8)))
    return np.ascontiguousarray(res.results[0]["out"].T).reshape(8, 8, 10)


# revision 21
# speedup vs baseline: 1.0051x; 1.0051x over previous
"""DTGNN Trainium2 Bass kernel (v2, latency-optimized).

Single-core algorithm (graph tiny: N=8, E=16), replicated across 8 NeuronCores
via SPMD; core 0's output returned. Optimizations vs v1:
  - bf16 matmul inputs for all wide matmuls (4x PE throughput); fp32 kept on
    the attention/softmax (alpha) path.
  - GAT attention reductions (h*a_s).sum folded into the X@W matmul as extra
    host-precomputed columns (W @ a_s); edge-attn term ae = ea @ (We@a_e).
  - CNN_2 tail (conv2+flatten+linear1+linear2) collapsed into 4 accumulating
    [33,10]x[33,64] matmuls with host-folded weights; conv1 bias commutes
    with maxpool and folds into the same constant row.
  - Biases folded as ones-row contraction tricks (no separate DVE adds).
  - Edge-MLP mean-pool (ef/er) computed as sums of the hidden layer pushed
    through (w2/8 | b2) on the PE.
  - 3 input DMAs (was 6), packed mixed-dtype via bf16-pair bitcasting.
"""
import numpy as np
import ml_dtypes
from contextlib import ExitStack

import concourse.bacc as bacc
import concourse.bass as bass
import concourse.tile as tile
import concourse.mybir as mybir
from concourse.bass_utils import run_bass_kernel_spmd

F32 = mybir.dt.float32
BF16 = mybir.dt.bfloat16
I32 = mybir.dt.int32
ALU = mybir.AluOpType
ACT = mybir.ActivationFunctionType
AXL = mybir.AxisListType


def _mkoff(lst):
    d, o = {}, 0
    for name, w in lst:
        d[name] = o
        o += w
    d["_W"] = o
    return d


# tA [33, *] f32 — constants + CNN1 + one-hot bits (lands first)
_LA = [("W1s", 8), ("TPAD3", 20), ("w2T", 3), ("mask16", 8), ("M24x", 24),
       ("iota_row24", 8), ("iota8", 1), ("ipack", 50), ("c1b1", 1),
       ("c1b2", 1), ("e17c", 1), ("ident16f", 16), ("ident8b", 4)]
# tB [128, *] f32 — GAT1 h weights + alpha1 + MLP (lands second)
_LB = [("XTb", 16), ("XTf", 32), ("G1b", 512), ("G1f", 32), ("W16t", 128),
       ("W16as", 8), ("eaT24", 24), ("Wae", 4), ("eaTb", 8), ("mlpw1b", 32),
       ("mlpb1", 1), ("wv2", 1)]
# tC [128, *] f32 — GAT2 + deconv + CNN2 weights (lands third)
_LC = [("G2b", 66), ("mlpw2e", 64), ("D1b", 320), ("D2b", 320),
       ("c2w1T", 48), ("A33", 20), ("g1brow", 128), ("g2brow", 64),
       ("cidx68", 4), ("pcol68", 1)]

_oA, _oB, _oC = _mkoff(_LA), _mkoff(_LB), _mkoff(_LC)
_WA = ((_oA["_W"] + 127) // 128) * 128
_WB = _oB["_W"]
_WC = _oC["_W"]


def _build_nc(stage=99):
    nc = bacc.Bacc("TRN2", target_bir_lowering=False)

    mA = nc.dram_tensor("mA", [33, _WA], F32, kind="ExternalInput")
    mB = nc.dram_tensor("mB", [128, _WB], F32, kind="ExternalInput")
    mC = nc.dram_tensor("mC", [128, _WC], F32, kind="ExternalInput")
    out = nc.dram_tensor("out", [10, 64], F32, kind="ExternalOutput")

    with tile.TileContext(nc) as tc, ExitStack() as ctx:
        sb = ctx.enter_context(tc.tile_pool(name="sb", bufs=1))
        ps = ctx.enter_context(tc.tile_pool(name="ps", bufs=5, space="PSUM"))
        pst = ctx.enter_context(tc.tile_pool(name="pst", bufs=3, space="PSUM"))
        ctx.enter_context(nc.allow_low_precision(reason="bf16 kernel"))

        def _go():
            # --------------------------------------------- input DMAs
            tA = sb.tile([33, _WA], F32)
            nc.sync.dma_start(tA[:], mA[:])
            tB = sb.tile([128, _WB], F32)
            nc.sync.dma_start(tB[:], mB[:])
            tC = sb.tile([128, _WC], F32)
            nc.sync.dma_start(tC[:], mC[:])

            def A(name, w, rows, r0=0):
                return tA[r0:r0 + rows, _oA[name]:_oA[name] + w]

            def Bv(name, w, rows, r0=0):
                return tB[r0:r0 + rows, _oB[name]:_oB[name] + w]

            def C(name, w, rows, r0=0):
                return tC[r0:r0 + rows, _oC[name]:_oC[name] + w]

            W1s = A("W1s", 8, 24)
            TPAD3 = A("TPAD3", 20, 24).rearrange("p (b t) -> p b t", b=2)
            w2T = A("w2T", 3, 10)
            mask16 = A("mask16", 8, 16)
            M24x = A("M24x", 24, 16)
            iota_row24 = A("iota_row24", 8, 24)
            iota8 = A("iota8", 1, 8)
            c1b1 = A("c1b1", 1, 8)
            c1b2 = A("c1b2", 1, 1)
            e17c = A("e17c", 1, 1)
            ident16f = A("ident16f", 16, 16)
            ident8b = A("ident8b", 4, 8).bitcast(BF16)

            XTb = Bv("XTb", 16, 128).bitcast(BF16).rearrange("p (j n) -> p j n", j=4)
            XTf = Bv("XTf", 32, 128).rearrange("p (j n) -> p j n", j=4)
            G1b = Bv("G1b", 512, 128).bitcast(BF16).rearrange("p (j n) -> p j n", j=4)
            G1f = Bv("G1f", 32, 128).rearrange("p (j n) -> p j n", j=4)
            W16t = Bv("W16t", 128, 16).bitcast(BF16)
            W16as = Bv("W16as", 8, 16)
            eaT24 = Bv("eaT24", 24, 128)
            Wae = Bv("Wae", 4, 128)
            eaTb = Bv("eaTb", 8, 128).bitcast(BF16)
            mlpw1b = Bv("mlpw1b", 32, 128).bitcast(BF16)
            mlpb1 = Bv("mlpb1", 1, 64)
            wv2 = Bv("wv2", 1, 64)

            G2b = C("G2b", 66, 128).bitcast(BF16).rearrange("p (j n) -> p j n", j=2)
            mlpw2e = C("mlpw2e", 64, 65)
            D1b = C("D1b", 320, 128).bitcast(BF16)
            D2b = C("D2b", 320, 68).bitcast(BF16)
            c2w1T = C("c2w1T", 48, 4).bitcast(BF16).rearrange("p (k n) -> p k n", k=3)
            A33 = C("A33", 20, 33).bitcast(BF16).rearrange("p (j n) -> p j n", j=4)
            g1brow = C("g1brow", 128, 1).bitcast(BF16)
            cidx68 = C("cidx68", 4, 68)
            pcol68 = C("pcol68", 1, 68)
            g2brow = C("g2brow", 64, 1)

            # --------------------------------------------- early memsets
            zp = sb.tile([10, 2, 10], F32)
            nc.vector.memset(zp[:], 0.0)
            warm = sb.tile([1, 1], F32)
            nc.vector.memset(warm[:], 0.0)
            warm2 = sb.tile([1, 1], F32)
            nc.scalar.activation(warm2[:], warm[:], ACT.Exp)
            sel = sb.tile([128, 4], BF16)
            nc.vector.memset(sel[:], 0.0)
            selL = sb.tile([64, 4], BF16)
            nc.vector.memset(selL[:], 0.0)
            sel2 = sb.tile([68, 4], BF16)
            Pdst24b = sb.tile([24, 8], BF16)
            y1c33 = sb.tile([33, 64, 4], BF16)
            nc.vector.memset(y1c33[32:33, :, :], 1.0)
            s2 = sb.tile([65, 2], F32)
            nc.vector.memset(s2[64:65, :], 1.0)
            ones1x8 = sb.tile([1, 8], F32)
            nc.vector.memset(ones1x8[:], 1.0)
            ones8bb = sb.tile([1, 8], BF16)
            nc.vector.memset(ones8bb[:], 1.0)
            ones24f = sb.tile([1, 24], F32)
            nc.vector.memset(ones24f[:], 1.0)
            ones8b = sb.tile([8, 1], F32)
            nc.vector.memset(ones8b[:], 0.125)

            # --------------------------------------------- one-hot matrices
            ti = A("ipack", 50, 24).bitcast(I32)
            tif = sb.tile([24, 50], F32)
            nc.vector.tensor_copy(tif[:], ti)
            idx_f = tif[0:8, 0:48].rearrange("p (c e) -> p c e", c=2)
            dcol_f = tif[:, 48:49]

            PsrcTf = sb.tile([8, 24], F32)
            nc.vector.tensor_scalar(PsrcTf[:], idx_f[:, 0, :], iota8, None,
                                    ALU.is_equal)
            PsrcTb = sb.tile([8, 24], BF16)
            nc.vector.tensor_scalar(PsrcTb[:], idx_f[:, 0, :], iota8, None,
                                    ALU.is_equal)
            PdstTf = sb.tile([8, 24], F32)
            nc.vector.tensor_scalar(PdstTf[:], idx_f[:, 1, :], iota8, None,
                                    ALU.is_equal)
            Pdst24f = sb.tile([24, 8], F32)
            nc.vector.tensor_scalar(Pdst24f[:], iota_row24, dcol_f, None,
                                    ALU.is_equal)
            nc.vector.tensor_scalar(Pdst24b[:], iota_row24, dcol_f, None,
                                    ALU.is_equal)

            # --------------------------------------------- CNN_1
            ps_y1 = ps.tile([8, 2, 10], F32, tag="ps")
            nc.tensor.matmul(ps_y1[:], W1s, TPAD3, start=True, stop=True)
            y1 = sb.tile([8, 2, 10], F32)
            nc.vector.tensor_scalar(y1[:], ps_y1[:], c1b1, 0.0, ALU.add, ALU.max)

            ps_za = ps.tile([10, 8], F32, tag="ps")
            nc.tensor.transpose(ps_za[:], y1[:, 0, :], ident16f[0:8, 0:8])
            ps_zb = ps.tile([10, 8], F32, tag="ps")
            nc.tensor.transpose(ps_zb[:], y1[:, 1, :], ident16f[0:8, 0:8])
            nc.vector.tensor_copy(zp[:, 0, 1:9], ps_za[:])
            nc.scalar.copy(zp[:, 1, 1:9], ps_zb[:])

            ps_y2 = ps.tile([1, 16], F32, tag="ps")
            for k in range(3):
                nc.tensor.matmul(ps_y2[:], w2T[:, k:k + 1], zp[:, :, k:k + 8],
                                 start=(k == 0), stop=(k == 2))
            xr = sb.tile([1, 16], F32)
            nc.vector.tensor_scalar(xr[:].rearrange("p (c n) -> p n c", c=2),
                                    ps_y2[:].rearrange("p (n c) -> p n c", c=2),
                                    c1b2, 0.0, ALU.add, ALU.max)
            ps_xrT = ps.tile([16, 1], F32, tag="ps")
            nc.tensor.transpose(ps_xrT[:], xr[:], ident16f[0:1, 0:1])
            x16f = sb.tile([16, 8], F32)
            nc.vector.tensor_tensor(x16f[:], ps_xrT[:].broadcast_to([16, 8]),
                                    mask16, ALU.mult)
            x16b = sb.tile([16, 8], BF16)
            nc.vector.tensor_tensor(x16b[:], ps_xrT[:].broadcast_to([16, 8]),
                                    mask16, ALU.mult)
            if stage == 1:
                o10 = sb.tile([10, 64], F32)
                nc.vector.memset(o10[:], 0.0)
                nc.vector.tensor_copy(o10[0:8, 0:20],
                                      y1[:].rearrange("p b t -> p (b t)"))
                nc.sync.dma_start(out[:], o10[:])
                return

            # --------------------------------------------- GAT 1
            ps_hf = ps.tile([8, 8], F32, tag="ps")
            for j in range(4):
                nc.tensor.matmul(ps_hf[:], XTf[:, j, :], G1f[:, j, :],
                                 start=(j == 0), stop=False)
            nc.tensor.matmul(ps_hf[:], x16f[:], W16as, start=False, stop=True)
            ps_h = ps.tile([8, 256], F32, tag="ps")
            for j in range(4):
                nc.tensor.matmul(ps_h[:], XTb[:, j, :], G1b[:, j, :],
                                 start=(j == 0), stop=False)
            nc.tensor.matmul(ps_h[:], x16b[:], W16t, start=False, stop=True)

            asad = sb.tile([8, 8], F32)
            nc.vector.tensor_copy(asad[:], ps_hf[:])
            h_sb = sb.tile([8, 256], BF16)
            nc.vector.tensor_copy(h_sb[:], ps_h[:])

            if stage == 21:
                o10 = sb.tile([10, 64], F32)
                nc.vector.memset(o10[:], 0.0)
                nc.vector.tensor_copy(o10[0:8, 0:56], ps_h[:, 0:56])
                nc.vector.tensor_copy(o10[0:8, 56:64], asad[:])
                nc.sync.dma_start(out[:], o10[:])
                return

            ps_al = ps.tile([24, 4], F32, tag="ps")
            nc.tensor.matmul(ps_al[:], eaT24, Wae, start=True, stop=False)
            nc.tensor.matmul(ps_al[:], PsrcTf[:], asad[:, 0:4], start=False,
                             stop=False)
            nc.tensor.matmul(ps_al[:], PdstTf[:], asad[:, 4:8], start=False,
                             stop=True)
            al1 = sb.tile([24, 4], F32)
            nc.vector.tensor_copy(al1[:], ps_al[:])
            ps_sg = ps.tile([24, 256], F32, tag="ps")
            nc.tensor.matmul(ps_sg[:], PsrcTb[:], h_sb[:], start=True, stop=True)
            lr1 = sb.tile([24, 4], F32)
            nc.vector.scalar_tensor_tensor(lr1[:], al1[:], 0.2, al1[:],
                                           ALU.mult, ALU.max)
            ex24 = sb.tile([24, 4], F32)
            nc.scalar.activation(ex24[:], lr1[:], ACT.Exp)
            ps_den = ps.tile([8, 4], F32, tag="ps")
            nc.tensor.matmul(ps_den[:], Pdst24f[:], ex24[:], start=True, stop=True)
            rden = sb.tile([8, 4], F32)
            nc.vector.reciprocal(rden[:], ps_den[:])
            ps_rde = ps.tile([24, 4], F32, tag="ps")
            nc.tensor.matmul(ps_rde[:], PdstTf[:], rden[:], start=True, stop=True)
            wexp = sb.tile([24, 4], F32)
            nc.vector.tensor_tensor(wexp[:], ex24[:], ps_rde[:], ALU.mult)

            if stage == 22:
                o10 = sb.tile([10, 64], F32)
                nc.vector.memset(o10[:], 0.0)
                nc.vector.tensor_copy(o10[0:8, 0:4], rden[:])
                nc.vector.tensor_copy(o10[0:8, 8:12], ps_den[:])
                nc.sync.dma_start(out[:], o10[:])
                return
            wh24 = sb.tile([24, 256], BF16)
            nc.vector.tensor_tensor(
                wh24[:].rearrange("p (h c) -> p h c", h=4),
                ps_sg[:].rearrange("p (h c) -> p h c", h=4),
                wexp[:].broadcast_to([24, 4, 64]), ALU.mult)
            ps_x1 = ps.tile([8, 256], F32, tag="ps")
            nc.tensor.matmul(ps_x1[:], ones8bb[:], g1brow, start=True, stop=False)
            nc.tensor.matmul(ps_x1[:], Pdst24b[:], wh24[:], start=False, stop=True)
            x1 = sb.tile([8, 256], BF16)
            nc.vector.tensor_scalar(x1[:], ps_x1[:], 0.0, None, ALU.max)
            if stage == 2:
                o10 = sb.tile([10, 64], F32)
                nc.vector.memset(o10[:], 0.0)
                nc.vector.tensor_copy(o10[0:8, 0:64], x1[:, 0:64])
                nc.sync.dma_start(out[:], o10[:])
                return

            nc.vector.tensor_scalar(sel2[:], cidx68, pcol68, None, ALU.is_equal)
            # --------------------------------------------- edge MLP (early)
            ps_m1 = ps.tile([64, 16], F32, tag="ps")
            nc.tensor.matmul(ps_m1[:], mlpw1b, eaTb, start=True, stop=True)
            r1T = sb.tile([64, 16], F32)
            nc.vector.tensor_scalar(r1T[:], ps_m1[:], mlpb1, 0.0, ALU.add,
                                    ALU.max)
            r1v = r1T[:].rearrange("p (e two) -> p two e", two=2)
            nc.vector.tensor_reduce(s2[0:64, 0:1], r1v[:, 0, :], axis=AXL.X,
                                    op=ALU.add)
            nc.vector.tensor_reduce(s2[0:64, 1:2], r1v[:, 1, :], axis=AXL.X,
                                    op=ALU.add)
            # ef into partitions 64:128 (for sel), er into 0:64 (for sel2)
            ps_ee = ps.tile([128, 2], F32, tag="ps")
            nc.tensor.matmul(ps_ee[64:128, 0:1], mlpw2e, s2[:, 0:1], start=True,
                             stop=True)
            nc.tensor.matmul(ps_ee[0:64, 1:2], mlpw2e, s2[:, 1:2], start=True,
                             stop=True)
            nc.vector.tensor_copy(sel[64:128, 2:3], ps_ee[64:128, 0:1])
            nc.vector.tensor_copy(sel2[0:64, 3:4], ps_ee[0:64, 1:2])

            ps_e16 = ps.tile([16, 1], F32, tag="ps")
            nc.tensor.matmul(ps_e16[:], r1T[:], wv2, start=True, stop=True)
            e16sb = sb.tile([16, 1], F32)
            nc.vector.tensor_copy(e16sb[:], ps_e16[:])
            b8sb = sb.tile([8, 64], F32)
            ps_b8 = ps.tile([8, 64], F32, tag="ps")
            nc.tensor.matmul(ps_b8[:], ones1x8[:], g2brow, start=True, stop=True)
            nc.vector.tensor_copy(b8sb[:], ps_b8[:])

            # --------------------------------------------- GAT 2
            ps_tr1 = ps.tile([128, 8], BF16, tag="ps")
            nc.tensor.transpose(ps_tr1[:], x1[:, 0:128], ident8b)
            ps_tr2 = ps.tile([128, 8], BF16, tag="ps")
            nc.tensor.transpose(ps_tr2[:], x1[:, 128:256], ident8b)
            x1T = sb.tile([128, 2, 8], BF16)
            nc.vector.tensor_copy(x1T[:, 0, :], ps_tr1[:])
            nc.vector.tensor_copy(x1T[:, 1, :], ps_tr2[:])

            ps_h2 = ps.tile([8, 66], F32, tag="ps")
            for j in range(2):
                nc.tensor.matmul(ps_h2[:], x1T[:, j, :], G2b[:, j, :],
                                 start=(j == 0), stop=(j == 1))
            hs2 = sb.tile([8, 66], F32)
            nc.vector.tensor_copy(hs2[:], ps_h2[:])

            ps_al2 = ps.tile([24, 1], F32, tag="ps")
            nc.tensor.matmul(ps_al2[:], ones24f[:], e17c, start=True, stop=False)
            nc.tensor.matmul(ps_al2[:], M24x, e16sb[:], start=False, stop=False)
            nc.tensor.matmul(ps_al2[:], PsrcTf[:], hs2[:, 64:65], start=False,
                             stop=False)
            nc.tensor.matmul(ps_al2[:], PdstTf[:], hs2[:, 65:66], start=False,
                             stop=True)
            al2 = sb.tile([24, 1], F32)
            nc.vector.tensor_copy(al2[:], ps_al2[:])
            lr2 = sb.tile([24, 1], F32)
            nc.vector.scalar_tensor_tensor(lr2[:], al2[:], 0.2, al2[:],
                                           ALU.mult, ALU.max)
            ex2 = sb.tile([24, 1], F32)
            nc.scalar.activation(ex2[:], lr2[:], ACT.Exp)
            ps_sg2 = ps.tile([24, 64], F32, tag="ps")
            nc.tensor.matmul(ps_sg2[:], PsrcTf[:], hs2[:, 0:64], start=True,
                             stop=True)
            ps_cTb = pst.tile([4, 320], F32, tag="pst")
            nc.tensor.matmul(ps_cTb[:], sel[:], D1b[:, 320:640], start=True,
                             stop=False)
            nc.tensor.matmul(ps_cTb[:], sel2[:], D2b[:, 320:640], start=False,
                             stop=False)
            ps_cTa = pst.tile([4, 320], F32, tag="pst")
            nc.tensor.matmul(ps_cTa[:], sel[:], D1b[:, 0:320], start=True,
                             stop=False)
            nc.tensor.matmul(ps_cTa[:], sel2[:], D2b[:, 0:320], start=False,
                             stop=False)
            ps_den2 = ps.tile([8, 1], F32, tag="ps")
            nc.tensor.matmul(ps_den2[:], Pdst24f[:], ex2[:], start=True, stop=True)
            rden2 = sb.tile([8, 1], F32)
            nc.vector.reciprocal(rden2[:], ps_den2[:])
            wh2 = sb.tile([24, 64], BF16)
            nc.vector.tensor_scalar(wh2[:], ps_sg2[:], ex2[:], None, ALU.mult)
            ps_x2u = ps.tile([8, 64], F32, tag="ps")
            nc.tensor.matmul(ps_x2u[:], Pdst24b[:], wh2[:], start=True,
                             stop=True)
            x2t = sb.tile([8, 64], F32)
            nc.vector.scalar_tensor_tensor(x2t[:], ps_x2u[:], rden2[:], b8sb[:],
                                           ALU.mult, ALU.add)
            x2 = sb.tile([8, 64], F32)
            nc.vector.tensor_scalar(x2[:], x2t[:], 0.0, None, ALU.max)
            if stage == 3:
                o10 = sb.tile([10, 64], F32)
                nc.vector.memset(o10[:], 0.0)
                nc.vector.tensor_copy(o10[0:8, 0:64], x2[:])
                nc.sync.dma_start(out[:], o10[:])
                return

            # --------------------------------------------- deconv selector
            ps_xm = ps.tile([64, 1], F32, tag="ps")
            nc.tensor.matmul(ps_xm[:], x2[:], ones8b[:], start=True, stop=True)
            nc.vector.tensor_copy(selL[0:64, 0:1], ps_xm[:])

            nc.tensor.matmul(ps_cTb[:], selL[:], D1b[0:64, 320:640],
                             start=False, stop=True)
            nc.tensor.matmul(ps_cTa[:], selL[:], D1b[0:64, 0:320],
                             start=False, stop=True)
            cT = sb.tile([4, 640], BF16)
            nc.scalar.copy(cT[:, 320:640], ps_cTb[:])
            nc.vector.tensor_copy(cT[:, 0:320], ps_cTa[:])
            if stage == 4:
                o10 = sb.tile([10, 64], F32)
                nc.vector.memset(o10[:], 0.0)
                nc.vector.tensor_copy(o10[0:4, 0:40], cT[:, 0:40])
                nc.sync.dma_start(out[:], o10[:])
                return

            # --------------------------------------------- CNN_2
            cTv = cT[:].rearrange("p (b l) -> p b l", b=64)
            ps_c1b = pst.tile([32, 32, 8], F32, tag="pst")
            for k in range(3):
                nc.tensor.matmul(ps_c1b[:], c2w1T[:, k, :],
                                 cTv[:, 32:64, k:k + 8],
                                 start=(k == 0), stop=(k == 2))
            ps_c1a = pst.tile([32, 32, 8], F32, tag="pst")
            for k in range(3):
                nc.tensor.matmul(ps_c1a[:], c2w1T[:, k, :],
                                 cTv[:, 0:32, k:k + 8],
                                 start=(k == 0), stop=(k == 2))
            nc.vector.tensor_reduce(
                y1c33[0:32, 32:64, :],
                ps_c1b[:].rearrange("p b (l two) -> p b l two", two=2),
                axis=AXL.X, op=ALU.max)
            nc.vector.tensor_reduce(
                y1c33[0:32, 0:32, :],
                ps_c1a[:].rearrange("p b (l two) -> p b l two", two=2),
                axis=AXL.X, op=ALU.max)

            ps_out = ps.tile([10, 64], F32, tag="ps")
            for j in range(4):
                nc.tensor.matmul(ps_out[:], A33[:, j, :], y1c33[:, :, j],
                                 start=(j == 0), stop=(j == 3))
            o10 = sb.tile([10, 64], F32)
            nc.vector.tensor_scalar(o10[:], ps_out[:], 0.0, None, ALU.max)
            nc.sync.dma_start(out[:], o10[:])

        _go()
    nc.finalize()
    return nc


_NC = None


def _get_nc():
    global _NC
    if _NC is None:
        _NC = _build_nc()
    return _NC


def _bfpack(a):
    """[r, c] float array -> [r, c/2] f32 whose bits hold bf16 pairs."""
    a = np.ascontiguousarray(np.asarray(a, dtype=np.float32))
    r, c = a.shape
    assert c % 2 == 0, c
    u = a.astype(ml_dtypes.bfloat16).view(np.uint16).reshape(r, c // 2, 2)
    packed = u[:, :, 0].astype(np.uint32) | (u[:, :, 1].astype(np.uint32) << 16)
    return packed.view(np.float32)


def _pack_inputs(x_feat, x_feat_tmp, edge_attr, c1w1, c1b1, c1w2, c1b2,
                 g1_lin, g1_as, g1_ad, g1_le, g1_ae, g1_b,
                 g2_lin, g2_as, g2_ad, g2_le, g2_ae, g2_b,
                 mlp_w1, mlp_b1, mlp_w2, mlp_b2,
                 d1w, d1b, d2w, d2b, d3w, d3b,
                 c2w1, c2b1, c2w2, c2b2, c2l1w, c2l1b, c2l2w, c2l2b,
                 edge_index):
    f = np.float32
    x_feat = np.asarray(x_feat, f)
    x_feat_tmp = np.asarray(x_feat_tmp, f)
    edge_attr = np.asarray(edge_attr, f)

    def fill(shape, off, blocks):
        arr = np.zeros(shape, dtype=f)
        for name, a in blocks.items():
            a = np.asarray(a, dtype=f)
            arr[0:a.shape[0], off[name]:off[name] + a.shape[1]] = a
        return arr

    # ---- tA ----
    tpad = np.zeros((8, 2, 12), dtype=f)
    for i in range(8):
        r = 1 if i % 2 == 0 else 5
        for b in range(2):
            tpad[i, b, 1:11] = x_feat_tmp[r, b * 4 + i // 2]
    T3 = np.zeros((24, 20), dtype=f)
    for k in range(3):
        for c in range(8):
            T3[k * 8 + c] = tpad[c, :, k:k + 10].reshape(20)
    W1s = np.asarray(c1w1, f).transpose(2, 1, 0).reshape(24, 8)

    mask16 = np.zeros((16, 8), dtype=f)
    for c in range(2):
        for n in range(8):
            mask16[c * 8 + n, n] = 1.0

    M24x = np.zeros((16, 24), dtype=f)
    M24x[0:16, 0:16] = np.eye(16, dtype=f)
    M24x[0:16, 16:24] = 1.0 / 16.0

    ve2 = np.asarray(g2_le, f) @ np.asarray(g2_ae, f).reshape(64)  # [64]
    wv2 = (np.asarray(mlp_w2, f) @ ve2).reshape(64, 1)
    e17c = float(np.asarray(mlp_b2, f) @ ve2)

    ipack = np.zeros((24, 50), dtype=np.int32)
    blk = np.zeros((8, 2, 24), dtype=np.int32)
    blk[:, :, 0:16] = np.asarray(edge_index, np.int32)[None, :, :]
    blk[:, :, 16:24] = np.arange(8, dtype=np.int32)[None, None, :]
    ipack[0:8, 0:48] = blk.reshape(8, 48)
    ipack[0:16, 48] = np.asarray(edge_index, np.int32)[1]
    ipack[16:24, 48] = np.arange(8, dtype=np.int32)

    tAm = fill((33, _WA), _oA, {
        "W1s": W1s, "TPAD3": T3,
        "w2T": np.asarray(c1w2, f).transpose(1, 2, 0).reshape(10, 3),
        "mask16": mask16, "M24x": M24x,
        "iota_row24": np.broadcast_to(np.arange(8, dtype=f), (24, 8)),
        "iota8": np.arange(8, dtype=f).reshape(8, 1),
        "ipack": ipack.view(np.float32),
        "c1b1": np.asarray(c1b1, f).reshape(8, 1),
        "c1b2": np.asarray(c1b2, f).reshape(1, 1),
        "ident16f": np.eye(16, dtype=f),
        "ident8b": _bfpack(np.eye(8, dtype=f)),
    })
    tAm[0, _oA["e17c"]] = e17c

    # ---- tB ----
    xfT = np.zeros((512, 8), dtype=f)
    xfT[0:510] = x_feat.T
    XT4 = xfT.reshape(4, 128, 8).transpose(1, 0, 2).reshape(128, 32)
    G1 = np.asarray(g1_lin, f)                       # [512, 256]
    Was = np.zeros((512, 4), dtype=f)
    Wad = np.zeros((512, 4), dtype=f)
    a_s = np.asarray(g1_as, f)
    a_d = np.asarray(g1_ad, f)
    for h in range(4):
        Was[:, h] = G1[:, h * 64:(h + 1) * 64] @ a_s[h]
        Wad[:, h] = G1[:, h * 64:(h + 1) * 64] @ a_d[h]
    G1f8 = np.concatenate([Was, Wad], 1)             # [512, 8]
    Wae = (np.asarray(g1_le, f).reshape(128, 4, 64) *
           np.asarray(g1_ae, f)[None]).sum(-1)       # [128, 4]
    ea_mean = edge_attr.mean(0)
    ea24 = np.concatenate([edge_attr, np.broadcast_to(ea_mean, (8, 128))], 0)

    tBm = fill((128, _WB), _oB, {
        "XTb": _bfpack(XT4),
        "XTf": XT4,
        "G1b": _bfpack(
            G1.reshape(4, 128, 256).transpose(1, 0, 2).reshape(128, 1024)),
        "G1f": G1f8.reshape(4, 128, 8).transpose(1, 0, 2).reshape(128, 32),
        "W16t": _bfpack(np.repeat(G1[510:512], 8, axis=0)),       # [16, 256]
        "W16as": np.repeat(G1f8[510:512], 8, axis=0),             # [16, 8]
        "eaT24": ea24.T,
        "Wae": Wae,
        "eaTb": _bfpack(edge_attr.T),
        "mlpw1b": _bfpack(np.asarray(mlp_w1, f)),
        "mlpb1": np.asarray(mlp_b1, f).reshape(64, 1),
        "wv2": wv2,
    })

    # ---- tC ----
    G2 = np.asarray(g2_lin, f)                        # [256, 64]
    was2 = (G2 @ np.asarray(g2_as, f).reshape(64)).reshape(256, 1)
    wad2 = (G2 @ np.asarray(g2_ad, f).reshape(64)).reshape(256, 1)
    G2e = np.concatenate([G2, was2, wad2], 1)         # [256, 66]
    G2e4 = G2e.reshape(2, 128, 66).transpose(1, 0, 2).reshape(128, 132)

    mlpw2e = np.zeros((65, 64), dtype=f)
    mlpw2e[0:64] = np.asarray(mlp_w2, f) * 0.125
    mlpw2e[64] = np.asarray(mlp_b2, f)

    D1 = np.concatenate([np.asarray(d1w, f).reshape(64, 640),
                         np.asarray(d2w, f).reshape(64, 640)], 0)
    b4 = np.zeros((4, 640), dtype=f)
    b4[0] = np.repeat(np.asarray(d1b, f), 10)
    b4[1] = x_feat_tmp.reshape(640)
    b4[2] = np.repeat(np.asarray(d2b, f), 10)
    b4[3] = np.repeat(np.asarray(d3b, f), 10)
    D2 = np.concatenate([np.asarray(d3w, f).reshape(64, 640), b4], 0)  # [68,640]

    # CNN2 fold: Wf [128,10] over (c2, l2); A[(c1,j), t]; const row.
    Wf = np.asarray(c2l1w, f) @ np.asarray(c2l2w, f)            # [128, 10]
    bfold = np.asarray(c2l1b, f) @ np.asarray(c2l2w, f) + np.asarray(c2l2b, f)
    WfR = Wf.reshape(64, 2, 10)                                 # [c2, l2, t]
    w2c = np.asarray(c2w2, f)                                   # [64, 32, 3]
    Afold = np.zeros((32, 4, 10), dtype=f)                      # [c1, j, t]
    for j in range(4):
        for l2 in range(2):
            k = j - l2
            if 0 <= k <= 2:
                Afold[:, j, :] += np.einsum("co,ct->ot", w2c[:, :, k],
                                            WfR[:, l2, :])
    const = (np.einsum("c,clt->t",
                       np.asarray(c2b2, f), WfR) + bfold +
             np.einsum("c,cjt->t", np.asarray(c2b1, f), Afold))
    A33m = np.zeros((33, 40), dtype=f)
    A33m[0:32] = Afold.reshape(32, 40)
    A33m[32, 0:10] = const
    tCm = fill((128, _WC), _oC, {
        "G2b": _bfpack(G2e4),
        "mlpw2e": mlpw2e,
        "D1b": _bfpack(D1),
        "D2b": _bfpack(D2),
        "c2w1T": _bfpack(np.asarray(c2w1, f).transpose(1, 2, 0).reshape(4, 96)),
        "A33": _bfpack(A33m),
        "g2brow": np.asarray(g2_b, f).reshape(1, 64),
        "g1brow": _bfpack(np.asarray(g1_b, f).reshape(1, 256)),
        "cidx68": np.broadcast_to(np.arange(4, dtype=f), (68, 4)),
        "pcol68": (np.arange(68, dtype=f) - 64.0).reshape(68, 1),
    })
    return tAm, tBm, tCm


def _make_ins(inputs):
    tAm, tBm, tCm = _pack_inputs(**inputs)
    return {"mA": tAm, "mB": tBm, "mC": tCm}


def kernel(**inputs):
    inputs = {k: np.ascontiguousarray(v) for k, v in inputs.items()}
    ins = _make_ins(inputs)
    nc = _get_nc()
    res = run_bass_kernel_spmd(nc, [ins] * 8, core_ids=list(range(8)))
    return np.ascontiguousarray(res.results[0]["out"].T).reshape(8, 8, 10)


# revision 23
# speedup vs baseline: 1.0094x; 1.0042x over previous
"""DTGNN Trainium2 Bass kernel (v2, latency-optimized).

Single-core algorithm (graph tiny: N=8, E=16), replicated across 8 NeuronCores
via SPMD; core 0's output returned. Optimizations vs v1:
  - bf16 matmul inputs for all wide matmuls (4x PE throughput); fp32 kept on
    the attention/softmax (alpha) path.
  - GAT attention reductions (h*a_s).sum folded into the X@W matmul as extra
    host-precomputed columns (W @ a_s); edge-attn term ae = ea @ (We@a_e).
  - CNN_2 tail (conv2+flatten+linear1+linear2) collapsed into 4 accumulating
    [33,10]x[33,64] matmuls with host-folded weights; conv1 bias commutes
    with maxpool and folds into the same constant row.
  - Biases folded as ones-row contraction tricks (no separate DVE adds).
  - Edge-MLP mean-pool (ef/er) computed as sums of the hidden layer pushed
    through (w2/8 | b2) on the PE.
  - 3 input DMAs (was 6), packed mixed-dtype via bf16-pair bitcasting.
"""
import numpy as np
import ml_dtypes
from contextlib import ExitStack

import concourse.bacc as bacc
import concourse.bass as bass
import concourse.tile as tile
import concourse.mybir as mybir
from concourse.bass_utils import run_bass_kernel_spmd

F32 = mybir.dt.float32
BF16 = mybir.dt.bfloat16
I32 = mybir.dt.int32
ALU = mybir.AluOpType
ACT = mybir.ActivationFunctionType
AXL = mybir.AxisListType


def _mkoff(lst):
    d, o = {}, 0
    for name, w in lst:
        d[name] = o
        o += w
    d["_W"] = o
    return d


# tA [33, *] f32 — constants + CNN1 + one-hot bits (lands first)
_LA = [("W1s", 8), ("TPAD3", 20), ("w2T", 3), ("mask16", 8), ("M24x", 24),
       ("iota_row24", 8), ("iota8", 1), ("ipack", 50), ("c1b1", 1),
       ("c1b2", 1), ("e17c", 1), ("ident16f", 16), ("ident8b", 4)]
# tB [128, *] f32 — GAT1 h weights + alpha1 + MLP (lands second)
_LB = [("XTb", 16), ("XTf", 32), ("G1b", 512), ("G1f", 32), ("W16t", 128),
       ("W16as", 8), ("eaT24", 24), ("Wae", 4), ("eaTb", 8), ("mlpw1b", 32),
       ("mlpb1", 1), ("wv2", 1)]
# tC [128, *] f32 — GAT2 + deconv + CNN2 weights (lands third)
_LC = [("G2b", 66), ("mlpw2e", 64), ("D1b", 320), ("D2b", 320),
       ("c2w1T", 48), ("A33", 20), ("g1brow", 128), ("g2brow", 64),
       ("cidx68", 4), ("pcol68", 1)]

_oA, _oB, _oC = _mkoff(_LA), _mkoff(_LB), _mkoff(_LC)
_WA = ((_oA["_W"] + 127) // 128) * 128
_WB = _oB["_W"]
_WC = _oC["_W"]


def _build_nc(stage=99):
    nc = bacc.Bacc("TRN2", target_bir_lowering=False)

    mA = nc.dram_tensor("mA", [33, _WA], F32, kind="ExternalInput")
    mB = nc.dram_tensor("mB", [128, _WB], F32, kind="ExternalInput")
    mC = nc.dram_tensor("mC", [128, _WC], F32, kind="ExternalInput")
    out = nc.dram_tensor("out", [10, 64], F32, kind="ExternalOutput")

    with tile.TileContext(nc) as tc, ExitStack() as ctx:
        sb = ctx.enter_context(tc.tile_pool(name="sb", bufs=1))
        ps = ctx.enter_context(tc.tile_pool(name="ps", bufs=5, space="PSUM"))
        pst = ctx.enter_context(tc.tile_pool(name="pst", bufs=3, space="PSUM"))
        ctx.enter_context(nc.allow_low_precision(reason="bf16 kernel"))

        def _go():
            # --------------------------------------------- input DMAs
            tA = sb.tile([33, _WA], F32)
            nc.sync.dma_start(tA[:], mA[:])
            tB = sb.tile([128, _WB], F32)
            nc.sync.dma_start(tB[:], mB[:])
            tC = sb.tile([128, _WC], F32)
            nc.sync.dma_start(tC[:], mC[:])

            def A(name, w, rows, r0=0):
                return tA[r0:r0 + rows, _oA[name]:_oA[name] + w]

            def Bv(name, w, rows, r0=0):
                return tB[r0:r0 + rows, _oB[name]:_oB[name] + w]

            def C(name, w, rows, r0=0):
                return tC[r0:r0 + rows, _oC[name]:_oC[name] + w]

            W1s = A("W1s", 8, 24)
            TPAD3 = A("TPAD3", 20, 24).rearrange("p (b t) -> p b t", b=2)
            w2T = A("w2T", 3, 10)
            mask16 = A("mask16", 8, 16)
            M24x = A("M24x", 24, 16)
            iota_row24 = A("iota_row24", 8, 24)
            iota8 = A("iota8", 1, 8)
            c1b1 = A("c1b1", 1, 8)
            c1b2 = A("c1b2", 1, 1)
            e17c = A("e17c", 1, 1)
            ident16f = A("ident16f", 16, 16)
            ident8b = A("ident8b", 4, 8).bitcast(BF16)

            XTb = Bv("XTb", 16, 128).bitcast(BF16).rearrange("p (j n) -> p j n", j=4)
            XTf = Bv("XTf", 32, 128).rearrange("p (j n) -> p j n", j=4)
            G1b = Bv("G1b", 512, 128).bitcast(BF16).rearrange("p (j n) -> p j n", j=4)
            G1f = Bv("G1f", 32, 128).rearrange("p (j n) -> p j n", j=4)
            W16t = Bv("W16t", 128, 16).bitcast(BF16)
            W16as = Bv("W16as", 8, 16)
            eaT24 = Bv("eaT24", 24, 128)
            Wae = Bv("Wae", 4, 128)
            eaTb = Bv("eaTb", 8, 128).bitcast(BF16)
            mlpw1b = Bv("mlpw1b", 32, 128).bitcast(BF16)
            mlpb1 = Bv("mlpb1", 1, 64)
            wv2 = Bv("wv2", 1, 64)

            G2b = C("G2b", 66, 128).bitcast(BF16).rearrange("p (j n) -> p j n", j=2)
            mlpw2e = C("mlpw2e", 64, 65)
            D1b = C("D1b", 320, 128).bitcast(BF16)
            D2b = C("D2b", 320, 68).bitcast(BF16)
            c2w1T = C("c2w1T", 48, 4).bitcast(BF16).rearrange("p (k n) -> p k n", k=3)
            A33 = C("A33", 20, 33).bitcast(BF16).rearrange("p (j n) -> p j n", j=4)
            g1brow = C("g1brow", 128, 1).bitcast(BF16)
            cidx68 = C("cidx68", 4, 68)
            pcol68 = C("pcol68", 1, 68)
            g2brow = C("g2brow", 64, 1)

            # --------------------------------------------- early memsets
            zp = sb.tile([10, 2, 10], F32)
            nc.vector.memset(zp[:], 0.0)
            warm = sb.tile([1, 1], F32)
            nc.vector.memset(warm[:], 0.0)
            warm2 = sb.tile([1, 1], F32)
            nc.scalar.activation(warm2[:], warm[:], ACT.Exp)
            sel = sb.tile([128, 4], BF16)
            nc.vector.memset(sel[:], 0.0)
            selL = sb.tile([64, 4], BF16)
            nc.vector.memset(selL[:], 0.0)
            sel2 = sb.tile([68, 4], BF16)
            Pdst24b = sb.tile([24, 8], BF16)
            y1c33 = sb.tile([33, 64, 4], BF16)
            nc.vector.memset(y1c33[32:33, :, :], 1.0)
            s2 = sb.tile([65, 2], F32)
            nc.vector.memset(s2[64:65, :], 1.0)
            ones1x8 = sb.tile([1, 8], F32)
            nc.vector.memset(ones1x8[:], 1.0)
            ones8bb = sb.tile([1, 8], BF16)
            nc.vector.memset(ones8bb[:], 1.0)
            ones24f = sb.tile([1, 24], F32)
            nc.vector.memset(ones24f[:], 1.0)
            ones8b = sb.tile([8, 1], F32)
            nc.vector.memset(ones8b[:], 0.125)

            # --------------------------------------------- one-hot matrices
            ti = A("ipack", 50, 24).bitcast(I32)
            tif = sb.tile([24, 50], F32)
            nc.vector.tensor_copy(tif[:], ti)
            idx_f = tif[0:8, 0:48].rearrange("p (c e) -> p c e", c=2)
            dcol_f = tif[:, 48:49]

            PsrcTf = sb.tile([8, 24], F32)
            nc.vector.tensor_scalar(PsrcTf[:], idx_f[:, 0, :], iota8, None,
                                    ALU.is_equal)
            PsrcTb = sb.tile([8, 24], BF16)
            nc.vector.tensor_scalar(PsrcTb[:], idx_f[:, 0, :], iota8, None,
                                    ALU.is_equal)
            PdstTf = sb.tile([8, 24], F32)
            nc.vector.tensor_scalar(PdstTf[:], idx_f[:, 1, :], iota8, None,
                                    ALU.is_equal)
            Pdst24f = sb.tile([24, 8], F32)
            nc.vector.tensor_scalar(Pdst24f[:], iota_row24, dcol_f, None,
                                    ALU.is_equal)
            nc.vector.tensor_scalar(Pdst24b[:], iota_row24, dcol_f, None,
                                    ALU.is_equal)

            # --------------------------------------------- CNN_1
            ps_y1 = ps.tile([8, 2, 10], F32, tag="ps")
            nc.tensor.matmul(ps_y1[:], W1s, TPAD3, start=True, stop=True)
            y1 = sb.tile([8, 2, 10], F32)
            nc.vector.tensor_scalar(y1[:], ps_y1[:], c1b1, 0.0, ALU.add, ALU.max)

            ps_za = ps.tile([10, 8], F32, tag="ps")
            nc.tensor.transpose(ps_za[:], y1[:, 0, :], ident16f[0:8, 0:8])
            ps_zb = ps.tile([10, 8], F32, tag="ps")
            nc.tensor.transpose(ps_zb[:], y1[:, 1, :], ident16f[0:8, 0:8])
            nc.vector.tensor_copy(zp[:, 0, 1:9], ps_za[:])
            nc.scalar.copy(zp[:, 1, 1:9], ps_zb[:])

            ps_y2 = ps.tile([1, 16], F32, tag="ps")
            for k in range(3):
                nc.tensor.matmul(ps_y2[:], w2T[:, k:k + 1], zp[:, :, k:k + 8],
                                 start=(k == 0), stop=(k == 2))
            xr = sb.tile([1, 16], F32)
            nc.vector.tensor_scalar(xr[:].rearrange("p (c n) -> p n c", c=2),
                                    ps_y2[:].rearrange("p (n c) -> p n c", c=2),
                                    c1b2, 0.0, ALU.add, ALU.max)
            ps_xrT = ps.tile([16, 1], F32, tag="ps")
            nc.tensor.transpose(ps_xrT[:], xr[:], ident16f[0:1, 0:1])
            x16f = sb.tile([16, 8], F32)
            nc.vector.tensor_tensor(x16f[:], ps_xrT[:].broadcast_to([16, 8]),
                                    mask16, ALU.mult)
            x16b = sb.tile([16, 8], BF16)
            nc.vector.tensor_tensor(x16b[:], ps_xrT[:].broadcast_to([16, 8]),
                                    mask16, ALU.mult)
            if stage == 1:
                o10 = sb.tile([10, 64], F32)
                nc.vector.memset(o10[:], 0.0)
                nc.vector.tensor_copy(o10[0:8, 0:20],
                                      y1[:].rearrange("p b t -> p (b t)"))
                nc.sync.dma_start(out[:], o10[:])
                return

            # --------------------------------------------- GAT 1
            ps_hf = ps.tile([8, 8], F32, tag="ps")
            for j in range(4):
                nc.tensor.matmul(ps_hf[:], XTf[:, j, :], G1f[:, j, :],
                                 start=(j == 0), stop=False)
            nc.tensor.matmul(ps_hf[:], x16f[:], W16as, start=False, stop=True)
            ps_h = ps.tile([8, 256], F32, tag="ps")
            for j in range(4):
                nc.tensor.matmul(ps_h[:], XTb[:, j, :], G1b[:, j, :],
                                 start=(j == 0), stop=False)
            nc.tensor.matmul(ps_h[:], x16b[:], W16t, start=False, stop=True)

            asad = sb.tile([8, 8], F32)
            nc.vector.tensor_copy(asad[:], ps_hf[:])
            h_sb = sb.tile([8, 256], BF16)
            nc.vector.tensor_copy(h_sb[:], ps_h[:])

            if stage == 21:
                o10 = sb.tile([10, 64], F32)
                nc.vector.memset(o10[:], 0.0)
                nc.vector.tensor_copy(o10[0:8, 0:56], ps_h[:, 0:56])
                nc.vector.tensor_copy(o10[0:8, 56:64], asad[:])
                nc.sync.dma_start(out[:], o10[:])
                return

            ps_al = ps.tile([24, 4], F32, tag="ps")
            nc.tensor.matmul(ps_al[:], eaT24, Wae, start=True, stop=False)
            nc.tensor.matmul(ps_al[:], PsrcTf[:], asad[:, 0:4], start=False,
                             stop=False)
            nc.tensor.matmul(ps_al[:], PdstTf[:], asad[:, 4:8], start=False,
                             stop=True)
            al1 = sb.tile([24, 4], F32)
            nc.vector.tensor_copy(al1[:], ps_al[:])
            ps_sg = ps.tile([24, 256], F32, tag="ps")
            nc.tensor.matmul(ps_sg[:], PsrcTb[:], h_sb[:], start=True, stop=True)
            lr1 = sb.tile([24, 4], F32)
            nc.vector.scalar_tensor_tensor(lr1[:], al1[:], 0.2, al1[:],
                                           ALU.mult, ALU.max)
            ex24 = sb.tile([24, 4], F32)
            nc.scalar.activation(ex24[:], lr1[:], ACT.Exp)
            ps_den = ps.tile([8, 4], F32, tag="ps")
            nc.tensor.matmul(ps_den[:], Pdst24f[:], ex24[:], start=True, stop=True)
            rden = sb.tile([8, 4], F32)
            nc.vector.reciprocal(rden[:], ps_den[:])
            ps_rde = ps.tile([24, 4], F32, tag="ps")
            nc.tensor.matmul(ps_rde[:], PdstTf[:], rden[:], start=True, stop=True)
            wexp = sb.tile([24, 4], F32)
            nc.vector.tensor_tensor(wexp[:], ex24[:], ps_rde[:], ALU.mult)

            if stage == 22:
                o10 = sb.tile([10, 64], F32)
                nc.vector.memset(o10[:], 0.0)
                nc.vector.tensor_copy(o10[0:8, 0:4], rden[:])
                nc.vector.tensor_copy(o10[0:8, 8:12], ps_den[:])
                nc.sync.dma_start(out[:], o10[:])
                return
            wh24 = sb.tile([24, 256], BF16)
            nc.vector.tensor_tensor(
                wh24[:].rearrange("p (h c) -> p h c", h=4),
                ps_sg[:].rearrange("p (h c) -> p h c", h=4),
                wexp[:].broadcast_to([24, 4, 64]), ALU.mult)
            ps_x1 = ps.tile([8, 256], F32, tag="ps")
            nc.tensor.matmul(ps_x1[:], ones8bb[:], g1brow, start=True, stop=False)
            nc.tensor.matmul(ps_x1[:], Pdst24b[:], wh24[:], start=False, stop=True)
            x1 = sb.tile([8, 256], BF16)
            nc.vector.tensor_scalar(x1[:], ps_x1[:], 0.0, None, ALU.max)
            if stage == 2:
                o10 = sb.tile([10, 64], F32)
                nc.vector.memset(o10[:], 0.0)
                nc.vector.tensor_copy(o10[0:8, 0:64], x1[:, 0:64])
                nc.sync.dma_start(out[:], o10[:])
                return

            nc.vector.tensor_scalar(sel2[:], cidx68, pcol68, None, ALU.is_equal)
            # --------------------------------------------- edge MLP (early)
            ps_m1 = ps.tile([64, 16], F32, tag="ps")
            nc.tensor.matmul(ps_m1[:], mlpw1b, eaTb, start=True, stop=True)
            r1T = sb.tile([64, 16], F32)
            nc.vector.tensor_scalar(r1T[:], ps_m1[:], mlpb1, 0.0, ALU.add,
                                    ALU.max)
            r1v = r1T[:].rearrange("p (e two) -> p two e", two=2)
            nc.vector.tensor_reduce(s2[0:64, 0:1], r1v[:, 0, :], axis=AXL.X,
                                    op=ALU.add)
            nc.vector.tensor_reduce(s2[0:64, 1:2], r1v[:, 1, :], axis=AXL.X,
                                    op=ALU.add)
            # ef into partitions 64:128 (for sel), er into 0:64 (for sel2)
            ps_ee = ps.tile([128, 2], F32, tag="ps")
            nc.tensor.matmul(ps_ee[64:128, 0:1], mlpw2e, s2[:, 0:1], start=True,
                             stop=True)
            nc.tensor.matmul(ps_ee[0:64, 1:2], mlpw2e, s2[:, 1:2], start=True,
                             stop=True)
            nc.vector.tensor_copy(sel[64:128, 2:3], ps_ee[64:128, 0:1])
            nc.vector.tensor_copy(sel2[0:64, 3:4], ps_ee[0:64, 1:2])

            ps_e16 = ps.tile([16, 1], F32, tag="ps")
            nc.tensor.matmul(ps_e16[:], r1T[:], wv2, start=True, stop=True)
            e16sb = sb.tile([16, 1], F32)
            nc.vector.tensor_copy(e16sb[:], ps_e16[:])
            b8sb = sb.tile([8, 64], F32)
            ps_b8 = ps.tile([8, 64], F32, tag="ps")
            nc.tensor.matmul(ps_b8[:], ones1x8[:], g2brow, start=True, stop=True)
            nc.vector.tensor_copy(b8sb[:], ps_b8[:])

            # --------------------------------------------- GAT 2
            ps_tr1 = ps.tile([128, 8], BF16, tag="ps")
            nc.tensor.transpose(ps_tr1[:], x1[:, 0:128], ident8b)
            ps_tr2 = ps.tile([128, 8], BF16, tag="ps")
            nc.tensor.transpose(ps_tr2[:], x1[:, 128:256], ident8b)
            x1T = sb.tile([128, 2, 8], BF16)
            nc.vector.tensor_copy(x1T[:, 0, :], ps_tr1[:])
            nc.vector.tensor_copy(x1T[:, 1, :], ps_tr2[:])

            ps_h2 = ps.tile([8, 66], F32, tag="ps")
            for j in range(2):
                nc.tensor.matmul(ps_h2[:], x1T[:, j, :], G2b[:, j, :],
                                 start=(j == 0), stop=(j == 1))
            hs2 = sb.tile([8, 66], F32)
            nc.vector.tensor_copy(hs2[:], ps_h2[:])

            ps_al2 = ps.tile([24, 1], F32, tag="ps")
            nc.tensor.matmul(ps_al2[:], ones24f[:], e17c, start=True, stop=False)
            nc.tensor.matmul(ps_al2[:], M24x, e16sb[:], start=False, stop=False)
            nc.tensor.matmul(ps_al2[:], PsrcTf[:], hs2[:, 64:65], start=False,
                             stop=False)
            nc.tensor.matmul(ps_al2[:], PdstTf[:], hs2[:, 65:66], start=False,
                             stop=True)
            al2 = sb.tile([24, 1], F32)
            nc.vector.tensor_copy(al2[:], ps_al2[:])
            lr2 = sb.tile([24, 1], F32)
            nc.vector.scalar_tensor_tensor(lr2[:], al2[:], 0.2, al2[:],
                                           ALU.mult, ALU.max)
            ex2 = sb.tile([24, 1], F32)
            nc.scalar.activation(ex2[:], lr2[:], ACT.Exp)
            ps_sg2 = ps.tile([24, 64], F32, tag="ps")
            nc.tensor.matmul(ps_sg2[:], PsrcTf[:], hs2[:, 0:64], start=True,
                             stop=True)
            ps_cTb = pst.tile([4, 320], F32, tag="pst")
            nc.tensor.matmul(ps_cTb[:], sel[:], D1b[:, 320:640], start=True,
                             stop=False)
            nc.tensor.matmul(ps_cTb[:], sel2[:], D2b[:, 320:640], start=False,
                             stop=False)
            ps_cTa = pst.tile([4, 320], F32, tag="pst")
            nc.tensor.matmul(ps_cTa[:], sel[:], D1b[:, 0:320], start=True,
                             stop=False)
            nc.tensor.matmul(ps_cTa[:], sel2[:], D2b[:, 0:320], start=False,
                             stop=False)
            ps_den2 = ps.tile([8, 1], F32, tag="ps")
            nc.tensor.matmul(ps_den2[:], Pdst24f[:], ex2[:], start=True, stop=True)
            rden2 = sb.tile([8, 1], F32)
            nc.vector.reciprocal(rden2[:], ps_den2[:])
            wh2 = sb.tile([24, 64], BF16)
            nc.vector.tensor_scalar(wh2[:], ps_sg2[:], ex2[:], None, ALU.mult)
            ps_x2u = ps.tile([8, 64], F32, tag="ps")
            nc.tensor.matmul(ps_x2u[:], Pdst24b[:], wh2[:], start=True,
                             stop=True)
            x2t = sb.tile([8, 64], F32)
            nc.vector.scalar_tensor_tensor(x2t[:], ps_x2u[:], rden2[:], b8sb[:],
                                           ALU.mult, ALU.add)
            x2 = sb.tile([8, 64], F32)
            nc.vector.tensor_scalar(x2[:], x2t[:], 0.0, None, ALU.max)
            if stage == 3:
                o10 = sb.tile([10, 64], F32)
                nc.vector.memset(o10[:], 0.0)
                nc.vector.tensor_copy(o10[0:8, 0:64], x2[:])
                nc.sync.dma_start(out[:], o10[:])
                return

            # --------------------------------------------- deconv selector
            ps_xm = ps.tile([64, 1], F32, tag="ps")
            nc.tensor.matmul(ps_xm[:], x2[:], ones8b[:], start=True, stop=True)
            nc.vector.tensor_copy(selL[0:64, 0:1], ps_xm[:])

            nc.tensor.matmul(ps_cTb[:], selL[:], D1b[0:64, 320:640],
                             start=False, stop=True)
            nc.tensor.matmul(ps_cTa[:], selL[:], D1b[0:64, 0:320],
                             start=False, stop=True)
            cT = sb.tile([4, 640], BF16)
            nc.scalar.copy(cT[:, 320:640], ps_cTb[:])
            nc.vector.tensor_copy(cT[:, 0:320], ps_cTa[:])
            if stage == 4:
                o10 = sb.tile([10, 64], F32)
                nc.vector.memset(o10[:], 0.0)
                nc.vector.tensor_copy(o10[0:4, 0:40], cT[:, 0:40])
                nc.sync.dma_start(out[:], o10[:])
                return

            # --------------------------------------------- CNN_2
            cTv = cT[:].rearrange("p (b l) -> p b l", b=64)
            ps_c1b = pst.tile([32, 32, 8], F32, tag="pst")
            for k in range(3):
                nc.tensor.matmul(ps_c1b[:], c2w1T[:, k, :],
                                 cTv[:, 32:64, k:k + 8],
                                 start=(k == 0), stop=(k == 2))
            ps_c1a = pst.tile([32, 32, 8], F32, tag="pst")
            for k in range(3):
                nc.tensor.matmul(ps_c1a[:], c2w1T[:, k, :],
                                 cTv[:, 0:32, k:k + 8],
                                 start=(k == 0), stop=(k == 2))
            nc.vector.tensor_reduce(
                y1c33[0:32, 32:64, :],
                ps_c1b[:].rearrange("p b (l two) -> p b l two", two=2),
                axis=AXL.X, op=ALU.max)
            nc.vector.tensor_reduce(
                y1c33[0:32, 0:32, :],
                ps_c1a[:].rearrange("p b (l two) -> p b l two", two=2),
                axis=AXL.X, op=ALU.max)

            o10 = sb.tile([10, 64], F32)
            ps_outB = ps.tile([10, 32], F32, tag="ps")
            for j in range(4):
                nc.tensor.matmul(ps_outB[:], A33[:, j, :], y1c33[:, 32:64, j],
                                 start=(j == 0), stop=(j == 3))
            nc.vector.tensor_scalar(o10[:, 32:64], ps_outB[:], 0.0, None,
                                    ALU.max)
            ps_outA = ps.tile([10, 32], F32, tag="ps")
            for j in range(4):
                nc.tensor.matmul(ps_outA[:], A33[:, j, :], y1c33[:, 0:32, j],
                                 start=(j == 0), stop=(j == 3))
            nc.vector.tensor_scalar(o10[:, 0:32], ps_outA[:], 0.0, None,
                                    ALU.max)
            nc.sync.dma_start(out[:], o10[:])

        _go()
    nc.finalize()
    return nc


_NC = None


def _get_nc():
    global _NC
    if _NC is None:
        _NC = _build_nc()
    return _NC


def _bfpack(a):
    """[r, c] float array -> [r, c/2] f32 whose bits hold bf16 pairs."""
    a = np.ascontiguousarray(np.asarray(a, dtype=np.float32))
    r, c = a.shape
    assert c % 2 == 0, c
    u = a.astype(ml_dtypes.bfloat16).view(np.uint16).reshape(r, c // 2, 2)
    packed = u[:, :, 0].astype(np.uint32) | (u[:, :, 1].astype(np.uint32) << 16)
    return packed.view(np.float32)


def _pack_inputs(x_feat, x_feat_tmp, edge_attr, c1w1, c1b1, c1w2, c1b2,
                 g1_lin, g1_as, g1_ad, g1_le, g1_ae, g1_b,
                 g2_lin, g2_as, g2_ad, g2_le, g2_ae, g2_b,
                 mlp_w1, mlp_b1, mlp_w2, mlp_b2,
                 d1w, d1b, d2w, d2b, d3w, d3b,
                 c2w1, c2b1, c2w2, c2b2, c2l1w, c2l1b, c2l2w, c2l2b,
                 edge_index):
    f = np.float32
    x_feat = np.asarray(x_feat, f)
    x_feat_tmp = np.asarray(x_feat_tmp, f)
    edge_attr = np.asarray(edge_attr, f)

    def fill(shape, off, blocks):
        arr = np.zeros(shape, dtype=f)
        for name, a in blocks.items():
            a = np.asarray(a, dtype=f)
            arr[0:a.shape[0], off[name]:off[name] + a.shape[1]] = a
        return arr

    # ---- tA ----
    tpad = np.zeros((8, 2, 12), dtype=f)
    for i in range(8):
        r = 1 if i % 2 == 0 else 5
        for b in range(2):
            tpad[i, b, 1:11] = x_feat_tmp[r, b * 4 + i // 2]
    T3 = np.zeros((24, 20), dtype=f)
    for k in range(3):
        for c in range(8):
            T3[k * 8 + c] = tpad[c, :, k:k + 10].reshape(20)
    W1s = np.asarray(c1w1, f).transpose(2, 1, 0).reshape(24, 8)

    mask16 = np.zeros((16, 8), dtype=f)
    for c in range(2):
        for n in range(8):
            mask16[c * 8 + n, n] = 1.0

    M24x = np.zeros((16, 24), dtype=f)
    M24x[0:16, 0:16] = np.eye(16, dtype=f)
    M24x[0:16, 16:24] = 1.0 / 16.0

    ve2 = np.asarray(g2_le, f) @ np.asarray(g2_ae, f).reshape(64)  # [64]
    wv2 = (np.asarray(mlp_w2, f) @ ve2).reshape(64, 1)
    e17c = float(np.asarray(mlp_b2, f) @ ve2)

    ipack = np.zeros((24, 50), dtype=np.int32)
    blk = np.zeros((8, 2, 24), dtype=np.int32)
    blk[:, :, 0:16] = np.asarray(edge_index, np.int32)[None, :, :]
    blk[:, :, 16:24] = np.arange(8, dtype=np.int32)[None, None, :]
    ipack[0:8, 0:48] = blk.reshape(8, 48)
    ipack[0:16, 48] = np.asarray(edge_index, np.int32)[1]
    ipack[16:24, 48] = np.arange(8, dtype=np.int32)

    tAm = fill((33, _WA), _oA, {
        "W1s": W1s, "TPAD3": T3,
        "w2T": np.asarray(c1w2, f).transpose(1, 2, 0).reshape(10, 3),
        "mask16": mask16, "M24x": M24x,
        "iota_row24": np.broadcast_to(np.arange(8, dtype=f), (24, 8)),
        "iota8": np.arange(8, dtype=f).reshape(8, 1),
        "ipack": ipack.view(np.float32),
        "c1b1": np.asarray(c1b1, f).reshape(8, 1),
        "c1b2": np.asarray(c1b2, f).reshape(1, 1),
        "ident16f": np.eye(16, dtype=f),
        "ident8b": _bfpack(np.eye(8, dtype=f)),
    })
    tAm[0, _oA["e17c"]] = e17c

    # ---- tB ----
    xfT = np.zeros((512, 8), dtype=f)
    xfT[0:510] = x_feat.T
    XT4 = xfT.reshape(4, 128, 8).transpose(1, 0, 2).reshape(128, 32)
    G1 = np.asarray(g1_lin, f)                       # [512, 256]
    Was = np.zeros((512, 4), dtype=f)
    Wad = np.zeros((512, 4), dtype=f)
    a_s = np.asarray(g1_as, f)
    a_d = np.asarray(g1_ad, f)
    for h in range(4):
        Was[:, h] = G1[:, h * 64:(h + 1) * 64] @ a_s[h]
        Wad[:, h] = G1[:, h * 64:(h + 1) * 64] @ a_d[h]
    G1f8 = np.concatenate([Was, Wad], 1)             # [512, 8]
    Wae = (np.asarray(g1_le, f).reshape(128, 4, 64) *
           np.asarray(g1_ae, f)[None]).sum(-1)       # [128, 4]
    ea_mean = edge_attr.mean(0)
    ea24 = np.concatenate([edge_attr, np.broadcast_to(ea_mean, (8, 128))], 0)

    tBm = fill((128, _WB), _oB, {
        "XTb": _bfpack(XT4),
        "XTf": XT4,
        "G1b": _bfpack(
            G1.reshape(4, 128, 256).transpose(1, 0, 2).reshape(128, 1024)),
        "G1f": G1f8.reshape(4, 128, 8).transpose(1, 0, 2).reshape(128, 32),
        "W16t": _bfpack(np.repeat(G1[510:512], 8, axis=0)),       # [16, 256]
        "W16as": np.repeat(G1f8[510:512], 8, axis=0),             # [16, 8]
        "eaT24": ea24.T,
        "Wae": Wae,
        "eaTb": _bfpack(edge_attr.T),
        "mlpw1b": _bfpack(np.asarray(mlp_w1, f)),
        "mlpb1": np.asarray(mlp_b1, f).reshape(64, 1),
        "wv2": wv2,
    })

    # ---- tC ----
    G2 = np.asarray(g2_lin, f)                        # [256, 64]
    was2 = (G2 @ np.asarray(g2_as, f).reshape(64)).reshape(256, 1)
    wad2 = (G2 @ np.asarray(g2_ad, f).reshape(64)).reshape(256, 1)
    G2e = np.concatenate([G2, was2, wad2], 1)         # [256, 66]
    G2e4 = G2e.reshape(2, 128, 66).transpose(1, 0, 2).reshape(128, 132)

    mlpw2e = np.zeros((65, 64), dtype=f)
    mlpw2e[0:64] = np.asarray(mlp_w2, f) * 0.125
    mlpw2e[64] = np.asarray(mlp_b2, f)

    D1 = np.concatenate([np.asarray(d1w, f).reshape(64, 640),
                         np.asarray(d2w, f).reshape(64, 640)], 0)
    b4 = np.zeros((4, 640), dtype=f)
    b4[0] = np.repeat(np.asarray(d1b, f), 10)
    b4[1] = x_feat_tmp.reshape(640)
    b4[2] = np.repeat(np.asarray(d2b, f), 10)
    b4[3] = np.repeat(np.asarray(d3b, f), 10)
    D2 = np.concatenate([np.asarray(d3w, f).reshape(64, 640), b4], 0)  # [68,640]

    # CNN2 fold: Wf [128,10] over (c2, l2); A[(c1,j), t]; const row.
    Wf = np.asarray(c2l1w, f) @ np.asarray(c2l2w, f)            # [128, 10]
    bfold = np.asarray(c2l1b, f) @ np.asarray(c2l2w, f) + np.asarray(c2l2b, f)
    WfR = Wf.reshape(64, 2, 10)                                 # [c2, l2, t]
    w2c = np.asarray(c2w2, f)                                   # [64, 32, 3]
    Afold = np.zeros((32, 4, 10), dtype=f)                      # [c1, j, t]
    for j in range(4):
        for l2 in range(2):
            k = j - l2
            if 0 <= k <= 2:
                Afold[:, j, :] += np.einsum("co,ct->ot", w2c[:, :, k],
                                            WfR[:, l2, :])
    const = (np.einsum("c,clt->t",
                       np.asarray(c2b2, f), WfR) + bfold +
             np.einsum("c,cjt->t", np.asarray(c2b1, f), Afold))
    A33m = np.zeros((33, 40), dtype=f)
    A33m[0:32] = Afold.reshape(32, 40)
    A33m[32, 0:10] = const
    tCm = fill((128, _WC), _oC, {
        "G2b": _bfpack(G2e4),
        "mlpw2e": mlpw2e,
        "D1b": _bfpack(D1),
        "D2b": _bfpack(D2),
        "c2w1T": _bfpack(np.asarray(c2w1, f).transpose(1, 2, 0).reshape(4, 96)),
        "A33": _bfpack(A33m),
        "g2brow": np.asarray(g2_b, f).reshape(1, 64),
        "g1brow": _bfpack(np.asarray(g1_b, f).reshape(1, 256)),
        "cidx68": np.broadcast_to(np.arange(4, dtype=f), (68, 4)),
        "pcol68": (np.arange(68, dtype=f) - 64.0).reshape(68, 1),
    })
    return tAm, tBm, tCm


def _make_ins(inputs):
    tAm, tBm, tCm = _pack_inputs(**inputs)
    return {"mA": tAm, "mB": tBm, "mC": tCm}


def kernel(**inputs):
    inputs = {k: np.ascontiguousarray(v) for k, v in inputs.items()}
    ins = _make_ins(inputs)
    nc = _get_nc()
    res = run_bass_kernel_spmd(nc, [ins] * 8, core_ids=list(range(8)))
    return np.ascontiguousarray(res.results[0]["out"].T).reshape(8, 8, 10)


# revision 25
# speedup vs baseline: 1.0335x; 1.0240x over previous
"""DTGNN Trainium2 Bass kernel (v2, latency-optimized).

Single-core algorithm (graph tiny: N=8, E=16), replicated across 8 NeuronCores
via SPMD; core 0's output returned. Optimizations vs v1:
  - bf16 matmul inputs for all wide matmuls (4x PE throughput); fp32 kept on
    the attention/softmax (alpha) path.
  - GAT attention reductions (h*a_s).sum folded into the X@W matmul as extra
    host-precomputed columns (W @ a_s); edge-attn term ae = ea @ (We@a_e).
  - CNN_2 tail (conv2+flatten+linear1+linear2) collapsed into 4 accumulating
    [33,10]x[33,64] matmuls with host-folded weights; conv1 bias commutes
    with maxpool and folds into the same constant row.
  - Biases folded as ones-row contraction tricks (no separate DVE adds).
  - Edge-MLP mean-pool (ef/er) computed as sums of the hidden layer pushed
    through (w2/8 | b2) on the PE.
  - 3 input DMAs (was 6), packed mixed-dtype via bf16-pair bitcasting.
"""
import numpy as np
import ml_dtypes
from contextlib import ExitStack

import concourse.bacc as bacc
import concourse.bass as bass
import concourse.tile as tile
import concourse.mybir as mybir
from concourse.bass_utils import run_bass_kernel_spmd

F32 = mybir.dt.float32
BF16 = mybir.dt.bfloat16
I32 = mybir.dt.int32
ALU = mybir.AluOpType
ACT = mybir.ActivationFunctionType
AXL = mybir.AxisListType


def _mkoff(lst):
    d, o = {}, 0
    for name, w in lst:
        d[name] = o
        o += w
    d["_W"] = o
    return d


# tA [33, *] f32 — constants + CNN1 + one-hot bits (lands first)
_LA = [("W1s", 8), ("TPAD3", 20), ("w2T", 3), ("mask16", 8), ("M24x", 24),
       ("iota_row24", 8), ("iota8", 1), ("ipack", 50), ("c1b1", 1),
       ("c1b2", 1), ("e17c", 1), ("ident16f", 16), ("ident8b", 4),
       ("c1b2x16", 1)]
# tB [128, *] f32 — GAT1 h weights + alpha1 + MLP (lands second)
_LB = [("XTb", 16), ("XTf", 32), ("G1b", 512), ("G1f", 32), ("W16t", 128),
       ("W16as", 8), ("eaT24", 24), ("Wae", 4), ("eaTb", 8), ("mlpw1b", 32),
       ("mlpb1", 1), ("wv2", 1)]
# tC [128, *] f32 — GAT2 + deconv + CNN2 weights (lands third)
_LC = [("G2b", 66), ("mlpw2e", 64), ("D1b", 320), ("D2b", 320),
       ("c2w1T", 48), ("A33", 20), ("g1brow", 128), ("g2brow", 64),
       ("cidx68", 4), ("pcol68", 1)]

_oA, _oB, _oC = _mkoff(_LA), _mkoff(_LB), _mkoff(_LC)
_WA = ((_oA["_W"] + 127) // 128) * 128
_WB = _oB["_W"]
_WC = _oC["_W"]


def _build_nc(stage=99):
    nc = bacc.Bacc("TRN2", target_bir_lowering=False)

    mA = nc.dram_tensor("mA", [33, _WA], F32, kind="ExternalInput")
    mB = nc.dram_tensor("mB", [128, _WB], F32, kind="ExternalInput")
    mC = nc.dram_tensor("mC", [128, _WC], F32, kind="ExternalInput")
    out = nc.dram_tensor("out", [10, 64], F32, kind="ExternalOutput")

    with tile.TileContext(nc) as tc, ExitStack() as ctx:
        sb = ctx.enter_context(tc.tile_pool(name="sb", bufs=1))
        ps = ctx.enter_context(tc.tile_pool(name="ps", bufs=5, space="PSUM"))
        pst = ctx.enter_context(tc.tile_pool(name="pst", bufs=3, space="PSUM"))
        ctx.enter_context(nc.allow_low_precision(reason="bf16 kernel"))

        def _go():
            # --------------------------------------------- input DMAs
            tA = sb.tile([33, _WA], F32)
            nc.sync.dma_start(tA[:], mA[:])
            tB = sb.tile([128, _WB], F32)
            nc.sync.dma_start(tB[:], mB[:])
            tC = sb.tile([128, _WC], F32)
            nc.sync.dma_start(tC[:], mC[:])

            def A(name, w, rows, r0=0):
                return tA[r0:r0 + rows, _oA[name]:_oA[name] + w]

            def Bv(name, w, rows, r0=0):
                return tB[r0:r0 + rows, _oB[name]:_oB[name] + w]

            def C(name, w, rows, r0=0):
                return tC[r0:r0 + rows, _oC[name]:_oC[name] + w]

            W1s = A("W1s", 8, 24)
            TPAD3 = A("TPAD3", 20, 24).rearrange("p (b t) -> p b t", b=2)
            w2T = A("w2T", 3, 10)
            mask16 = A("mask16", 8, 16)
            M24x = A("M24x", 24, 16)
            iota_row24 = A("iota_row24", 8, 24)
            iota8 = A("iota8", 1, 8)
            c1b1 = A("c1b1", 1, 8)
            c1b2x16 = A("c1b2x16", 1, 16)
            e17c = A("e17c", 1, 1)
            ident16f = A("ident16f", 16, 16)
            ident8b = A("ident8b", 4, 8).bitcast(BF16)

            XTb = Bv("XTb", 16, 128).bitcast(BF16).rearrange("p (j n) -> p j n", j=4)
            XTf = Bv("XTf", 32, 128).rearrange("p (j n) -> p j n", j=4)
            G1b = Bv("G1b", 512, 128).bitcast(BF16).rearrange("p (j n) -> p j n", j=4)
            G1f = Bv("G1f", 32, 128).rearrange("p (j n) -> p j n", j=4)
            W16t = Bv("W16t", 128, 16).bitcast(BF16)
            W16as = Bv("W16as", 8, 16)
            eaT24 = Bv("eaT24", 24, 128)
            Wae = Bv("Wae", 4, 128)
            eaTb = Bv("eaTb", 8, 128).bitcast(BF16)
            mlpw1b = Bv("mlpw1b", 32, 128).bitcast(BF16)
            mlpb1 = Bv("mlpb1", 1, 64)
            wv2 = Bv("wv2", 1, 64)

            G2b = C("G2b", 66, 128).bitcast(BF16).rearrange("p (j n) -> p j n", j=2)
            mlpw2e = C("mlpw2e", 64, 65)
            D1b = C("D1b", 320, 128).bitcast(BF16)
            D2b = C("D2b", 320, 68).bitcast(BF16)
            c2w1T = C("c2w1T", 48, 4).bitcast(BF16).rearrange("p (k n) -> p k n", k=3)
            A33 = C("A33", 20, 33).bitcast(BF16).rearrange("p (j n) -> p j n", j=4)
            g1brow = C("g1brow", 128, 1).bitcast(BF16)
            cidx68 = C("cidx68", 4, 68)
            pcol68 = C("pcol68", 1, 68)
            g2brow = C("g2brow", 64, 1)

            # --------------------------------------------- early memsets
            zp = sb.tile([10, 24], F32)
            nc.vector.memset(zp[:], 0.0)
            warm = sb.tile([1, 1], F32)
            nc.vector.memset(warm[:], 0.0)
            warm2 = sb.tile([1, 1], F32)
            nc.scalar.activation(warm2[:], warm[:], ACT.Exp)
            sel = sb.tile([128, 4], BF16)
            nc.vector.memset(sel[:], 0.0)
            selL = sb.tile([64, 4], BF16)
            nc.vector.memset(selL[:], 0.0)
            sel2 = sb.tile([68, 4], BF16)
            Pdst24b = sb.tile([24, 8], BF16)
            y1c33 = sb.tile([33, 64, 4], BF16)
            nc.vector.memset(y1c33[32:33, :, :], 1.0)
            s2 = sb.tile([65, 2], F32)
            nc.vector.memset(s2[64:65, :], 1.0)
            ones1x8 = sb.tile([1, 8], F32)
            nc.vector.memset(ones1x8[:], 1.0)
            ones8bb = sb.tile([1, 8], BF16)
            nc.vector.memset(ones8bb[:], 1.0)
            ones24f = sb.tile([1, 24], F32)
            nc.vector.memset(ones24f[:], 1.0)
            ones8b = sb.tile([8, 1], F32)
            nc.vector.memset(ones8b[:], 0.125)

            # --------------------------------------------- one-hot matrices
            ti = A("ipack", 50, 24).bitcast(I32)
            tif = sb.tile([24, 50], F32)
            nc.vector.tensor_copy(tif[:], ti)
            idx_f = tif[0:8, 0:48].rearrange("p (c e) -> p c e", c=2)
            dcol_f = tif[:, 48:49]

            PsrcTf = sb.tile([8, 24], F32)
            nc.vector.tensor_scalar(PsrcTf[:], idx_f[:, 0, :], iota8, None,
                                    ALU.is_equal)
            PsrcTb = sb.tile([8, 24], BF16)
            nc.vector.tensor_scalar(PsrcTb[:], idx_f[:, 0, :], iota8, None,
                                    ALU.is_equal)
            PdstTf = sb.tile([8, 24], F32)
            nc.vector.tensor_scalar(PdstTf[:], idx_f[:, 1, :], iota8, None,
                                    ALU.is_equal)
            Pdst24f = sb.tile([24, 8], F32)
            nc.vector.tensor_scalar(Pdst24f[:], iota_row24, dcol_f, None,
                                    ALU.is_equal)
            nc.vector.tensor_scalar(Pdst24b[:], iota_row24, dcol_f, None,
                                    ALU.is_equal)

            # --------------------------------------------- CNN_1
            ps_y1 = ps.tile([8, 2, 10], F32, tag="ps")
            nc.tensor.matmul(ps_y1[:], W1s, TPAD3, start=True, stop=True)
            y1 = sb.tile([8, 2, 10], F32)
            nc.vector.tensor_scalar(y1[:], ps_y1[:], c1b1, 0.0, ALU.add, ALU.max)

            ps_za = ps.tile([10, 8], F32, tag="ps")
            nc.tensor.transpose(ps_za[:], y1[:, 0, :], ident16f[0:8, 0:8])
            ps_zb = ps.tile([10, 8], F32, tag="ps")
            nc.tensor.transpose(ps_zb[:], y1[:, 1, :], ident16f[0:8, 0:8])
            zpv = zp[:].rearrange("p (t v) -> p t v", v=2)
            nc.vector.tensor_copy(zpv[:, 1:9, 0], ps_za[:])
            nc.scalar.copy(zpv[:, 1:9, 1], ps_zb[:])

            ps_y2T = ps.tile([16, 1], F32, tag="ps")
            for k in range(3):
                nc.tensor.matmul(ps_y2T[:], zp[:, 2 * k:2 * k + 16],
                                 w2T[:, k:k + 1],
                                 start=(k == 0), stop=(k == 2))
            xv16 = sb.tile([16, 1], F32)
            nc.vector.tensor_scalar(xv16[:], ps_y2T[:], c1b2x16, 0.0,
                                    ALU.add, ALU.max)
            x16f = sb.tile([16, 8], F32)
            nc.vector.tensor_tensor(x16f[:], xv16[:].broadcast_to([16, 8]),
                                    mask16, ALU.mult)
            x16b = sb.tile([16, 8], BF16)
            nc.vector.tensor_tensor(x16b[:], xv16[:].broadcast_to([16, 8]),
                                    mask16, ALU.mult)
            if stage == 1:
                o10 = sb.tile([10, 64], F32)
                nc.vector.memset(o10[:], 0.0)
                nc.vector.tensor_copy(o10[0:8, 0:20],
                                      y1[:].rearrange("p b t -> p (b t)"))
                nc.sync.dma_start(out[:], o10[:])
                return

            # --------------------------------------------- GAT 1
            ps_hf = ps.tile([8, 8], F32, tag="ps")
            for j in range(4):
                nc.tensor.matmul(ps_hf[:], XTf[:, j, :], G1f[:, j, :],
                                 start=(j == 0), stop=False)
            nc.tensor.matmul(ps_hf[:], x16f[:], W16as, start=False, stop=True)
            ps_h = ps.tile([8, 256], F32, tag="ps")
            for j in range(4):
                nc.tensor.matmul(ps_h[:], XTb[:, j, :], G1b[:, j, :],
                                 start=(j == 0), stop=False)
            nc.tensor.matmul(ps_h[:], x16b[:], W16t, start=False, stop=True)

            asad = sb.tile([8, 8], F32)
            nc.vector.tensor_copy(asad[:], ps_hf[:])
            h_sb = sb.tile([8, 256], BF16)
            nc.vector.tensor_copy(h_sb[:], ps_h[:])

            if stage == 21:
                o10 = sb.tile([10, 64], F32)
                nc.vector.memset(o10[:], 0.0)
                nc.vector.tensor_copy(o10[0:8, 0:56], ps_h[:, 0:56])
                nc.vector.tensor_copy(o10[0:8, 56:64], asad[:])
                nc.sync.dma_start(out[:], o10[:])
                return

            ps_al = ps.tile([24, 4], F32, tag="ps")
            nc.tensor.matmul(ps_al[:], eaT24, Wae, start=True, stop=False)
            nc.tensor.matmul(ps_al[:], PsrcTf[:], asad[:, 0:4], start=False,
                             stop=False)
            nc.tensor.matmul(ps_al[:], PdstTf[:], asad[:, 4:8], start=False,
                             stop=True)
            al1 = sb.tile([24, 4], F32)
            nc.vector.tensor_copy(al1[:], ps_al[:])
            ps_sg = ps.tile([24, 256], F32, tag="ps")
            nc.tensor.matmul(ps_sg[:], PsrcTb[:], h_sb[:], start=True, stop=True)
            lr1 = sb.tile([24, 4], F32)
            nc.vector.scalar_tensor_tensor(lr1[:], al1[:], 0.2, al1[:],
                                           ALU.mult, ALU.max)
            ex24 = sb.tile([24, 4], F32)
            nc.scalar.activation(ex24[:], lr1[:], ACT.Exp)
            ps_den = ps.tile([8, 4], F32, tag="ps")
            nc.tensor.matmul(ps_den[:], Pdst24f[:], ex24[:], start=True, stop=True)
            rden = sb.tile([8, 4], F32)
            nc.vector.reciprocal(rden[:], ps_den[:])
            ps_rde = ps.tile([24, 4], F32, tag="ps")
            nc.tensor.matmul(ps_rde[:], PdstTf[:], rden[:], start=True, stop=True)
            wexp = sb.tile([24, 4], F32)
            nc.vector.tensor_tensor(wexp[:], ex24[:], ps_rde[:], ALU.mult)

            if stage == 22:
                o10 = sb.tile([10, 64], F32)
                nc.vector.memset(o10[:], 0.0)
                nc.vector.tensor_copy(o10[0:8, 0:4], rden[:])
                nc.vector.tensor_copy(o10[0:8, 8:12], ps_den[:])
                nc.sync.dma_start(out[:], o10[:])
                return
            wh24 = sb.tile([24, 256], BF16)
            nc.vector.tensor_tensor(
                wh24[:].rearrange("p (h c) -> p h c", h=4),
                ps_sg[:].rearrange("p (h c) -> p h c", h=4),
                wexp[:].broadcast_to([24, 4, 64]), ALU.mult)
            ps_x1 = ps.tile([8, 256], F32, tag="ps")
            nc.tensor.matmul(ps_x1[:], ones8bb[:], g1brow, start=True, stop=False)
            nc.tensor.matmul(ps_x1[:], Pdst24b[:], wh24[:], start=False, stop=True)
            x1 = sb.tile([8, 256], BF16)
            nc.vector.tensor_scalar(x1[:], ps_x1[:], 0.0, None, ALU.max)
            if stage == 2:
                o10 = sb.tile([10, 64], F32)
                nc.vector.memset(o10[:], 0.0)
                nc.vector.tensor_copy(o10[0:8, 0:64], x1[:, 0:64])
                nc.sync.dma_start(out[:], o10[:])
                return

            nc.vector.tensor_scalar(sel2[:], cidx68, pcol68, None, ALU.is_equal)
            # --------------------------------------------- edge MLP (early)
            ps_m1 = ps.tile([64, 16], F32, tag="ps")
            nc.tensor.matmul(ps_m1[:], mlpw1b, eaTb, start=True, stop=True)
            r1T = sb.tile([64, 16], F32)
            nc.vector.tensor_scalar(r1T[:], ps_m1[:], mlpb1, 0.0, ALU.add,
                                    ALU.max)
            r1v = r1T[:].rearrange("p (e two) -> p two e", two=2)
            nc.vector.tensor_reduce(s2[0:64, 0:1], r1v[:, 0, :], axis=AXL.X,
                                    op=ALU.add)
            nc.vector.tensor_reduce(s2[0:64, 1:2], r1v[:, 1, :], axis=AXL.X,
                                    op=ALU.add)
            # ef into partitions 64:128 (for sel), er into 0:64 (for sel2)
            ps_ee = ps.tile([128, 2], F32, tag="ps")
            nc.tensor.matmul(ps_ee[64:128, 0:1], mlpw2e, s2[:, 0:1], start=True,
                             stop=True)
            nc.tensor.matmul(ps_ee[0:64, 1:2], mlpw2e, s2[:, 1:2], start=True,
                             stop=True)
            nc.vector.tensor_copy(sel[64:128, 2:3], ps_ee[64:128, 0:1])
            nc.vector.tensor_copy(sel2[0:64, 3:4], ps_ee[0:64, 1:2])

            ps_e16 = ps.tile([16, 1], F32, tag="ps")
            nc.tensor.matmul(ps_e16[:], r1T[:], wv2, start=True, stop=True)
            e16sb = sb.tile([16, 1], F32)
            nc.vector.tensor_copy(e16sb[:], ps_e16[:])
            b8sb = sb.tile([8, 64], F32)
            ps_b8 = ps.tile([8, 64], F32, tag="ps")
            nc.tensor.matmul(ps_b8[:], ones1x8[:], g2brow, start=True, stop=True)
            nc.vector.tensor_copy(b8sb[:], ps_b8[:])

            # --------------------------------------------- GAT 2
            ps_tr1 = ps.tile([128, 8], BF16, tag="ps")
            nc.tensor.transpose(ps_tr1[:], x1[:, 0:128], ident8b)
            ps_tr2 = ps.tile([128, 8], BF16, tag="ps")
            nc.tensor.transpose(ps_tr2[:], x1[:, 128:256], ident8b)
            x1T = sb.tile([128, 2, 8], BF16)
            nc.vector.tensor_copy(x1T[:, 0, :], ps_tr1[:])
            nc.vector.tensor_copy(x1T[:, 1, :], ps_tr2[:])

            ps_h2 = ps.tile([8, 66], F32, tag="ps")
            for j in range(2):
                nc.tensor.matmul(ps_h2[:], x1T[:, j, :], G2b[:, j, :],
                                 start=(j == 0), stop=(j == 1))
            hs2 = sb.tile([8, 66], F32)
            nc.vector.tensor_copy(hs2[:], ps_h2[:])

            ps_al2 = ps.tile([24, 1], F32, tag="ps")
            nc.tensor.matmul(ps_al2[:], ones24f[:], e17c, start=True, stop=False)
            nc.tensor.matmul(ps_al2[:], M24x, e16sb[:], start=False, stop=False)
            nc.tensor.matmul(ps_al2[:], PsrcTf[:], hs2[:, 64:65], start=False,
                             stop=False)
            nc.tensor.matmul(ps_al2[:], PdstTf[:], hs2[:, 65:66], start=False,
                             stop=True)
            al2 = sb.tile([24, 1], F32)
            nc.vector.tensor_copy(al2[:], ps_al2[:])
            lr2 = sb.tile([24, 1], F32)
            nc.vector.scalar_tensor_tensor(lr2[:], al2[:], 0.2, al2[:],
                                           ALU.mult, ALU.max)
            ex2 = sb.tile([24, 1], F32)
            nc.scalar.activation(ex2[:], lr2[:], ACT.Exp)
            ps_sg2 = ps.tile([24, 64], F32, tag="ps")
            nc.tensor.matmul(ps_sg2[:], PsrcTf[:], hs2[:, 0:64], start=True,
                             stop=True)
            ps_cTb = pst.tile([4, 320], F32, tag="pst")
            nc.tensor.matmul(ps_cTb[:], sel[:], D1b[:, 320:640], start=True,
                             stop=False)
            nc.tensor.matmul(ps_cTb[:], sel2[:], D2b[:, 320:640], start=False,
                             stop=False)
            ps_cTa = pst.tile([4, 320], F32, tag="pst")
            nc.tensor.matmul(ps_cTa[:], sel[:], D1b[:, 0:320], start=True,
                             stop=False)
            nc.tensor.matmul(ps_cTa[:], sel2[:], D2b[:, 0:320], start=False,
                             stop=False)
            ps_den2 = ps.tile([8, 1], F32, tag="ps")
            nc.tensor.matmul(ps_den2[:], Pdst24f[:], ex2[:], start=True, stop=True)
            rden2 = sb.tile([8, 1], F32)
            nc.vector.reciprocal(rden2[:], ps_den2[:])
            wh2 = sb.tile([24, 64], BF16)
            nc.vector.tensor_scalar(wh2[:], ps_sg2[:], ex2[:], None, ALU.mult)
            ps_x2u = ps.tile([8, 64], F32, tag="ps")
            nc.tensor.matmul(ps_x2u[:], Pdst24b[:], wh2[:], start=True,
                             stop=True)
            x2t = sb.tile([8, 64], F32)
            nc.vector.scalar_tensor_tensor(x2t[:], ps_x2u[:], rden2[:], b8sb[:],
                                           ALU.mult, ALU.add)
            x2 = sb.tile([8, 64], F32)
            nc.vector.tensor_scalar(x2[:], x2t[:], 0.0, None, ALU.max)
            if stage == 3:
                o10 = sb.tile([10, 64], F32)
                nc.vector.memset(o10[:], 0.0)
                nc.vector.tensor_copy(o10[0:8, 0:64], x2[:])
                nc.sync.dma_start(out[:], o10[:])
                return

            # --------------------------------------------- deconv selector
            ps_xm = ps.tile([64, 1], F32, tag="ps")
            nc.tensor.matmul(ps_xm[:], x2[:], ones8b[:], start=True, stop=True)
            nc.vector.tensor_copy(selL[0:64, 0:1], ps_xm[:])

            nc.tensor.matmul(ps_cTb[:], selL[:], D1b[0:64, 320:640],
                             start=False, stop=True)
            nc.tensor.matmul(ps_cTa[:], selL[:], D1b[0:64, 0:320],
                             start=False, stop=True)
            cT = sb.tile([4, 640], BF16)
            nc.scalar.copy(cT[:, 320:640], ps_cTb[:])
            nc.vector.tensor_copy(cT[:, 0:320], ps_cTa[:])
            if stage == 4:
                o10 = sb.tile([10, 64], F32)
                nc.vector.memset(o10[:], 0.0)
                nc.vector.tensor_copy(o10[0:4, 0:40], cT[:, 0:40])
                nc.sync.dma_start(out[:], o10[:])
                return

            # --------------------------------------------- CNN_2
            cTv = cT[:].rearrange("p (b l) -> p b l", b=64)
            ps_c1b = pst.tile([32, 32, 8], F32, tag="pst")
            for k in range(3):
                nc.tensor.matmul(ps_c1b[:], c2w1T[:, k, :],
                                 cTv[:, 32:64, k:k + 8],
                                 start=(k == 0), stop=(k == 2))
            ps_c1a = pst.tile([32, 32, 8], F32, tag="pst")
            for k in range(3):
                nc.tensor.matmul(ps_c1a[:], c2w1T[:, k, :],
                                 cTv[:, 0:32, k:k + 8],
                                 start=(k == 0), stop=(k == 2))
            nc.vector.tensor_reduce(
                y1c33[0:32, 32:64, :],
                ps_c1b[:].rearrange("p b (l two) -> p b l two", two=2),
                axis=AXL.X, op=ALU.max)
            nc.vector.tensor_reduce(
                y1c33[0:32, 0:32, :],
                ps_c1a[:].rearrange("p b (l two) -> p b l two", two=2),
                axis=AXL.X, op=ALU.max)

            o10 = sb.tile([10, 64], F32)
            ps_outB = ps.tile([10, 32], F32, tag="ps")
            for j in range(4):
                nc.tensor.matmul(ps_outB[:], A33[:, j, :], y1c33[:, 32:64, j],
                                 start=(j == 0), stop=(j == 3))
            nc.vector.tensor_scalar(o10[:, 32:64], ps_outB[:], 0.0, None,
                                    ALU.max)
            ps_outA = ps.tile([10, 32], F32, tag="ps")
            for j in range(4):
                nc.tensor.matmul(ps_outA[:], A33[:, j, :], y1c33[:, 0:32, j],
                                 start=(j == 0), stop=(j == 3))
            nc.vector.tensor_scalar(o10[:, 0:32], ps_outA[:], 0.0, None,
                                    ALU.max)
            nc.sync.dma_start(out[:], o10[:])

        _go()
    nc.finalize()
    return nc


_NC = None


def _get_nc():
    global _NC
    if _NC is None:
        _NC = _build_nc()
    return _NC


def _bfpack(a):
    """[r, c] float array -> [r, c/2] f32 whose bits hold bf16 pairs."""
    a = np.ascontiguousarray(np.asarray(a, dtype=np.float32))
    r, c = a.shape
    assert c % 2 == 0, c
    u = a.astype(ml_dtypes.bfloat16).view(np.uint16).reshape(r, c // 2, 2)
    packed = u[:, :, 0].astype(np.uint32) | (u[:, :, 1].astype(np.uint32) << 16)
    return packed.view(np.float32)


def _pack_inputs(x_feat, x_feat_tmp, edge_attr, c1w1, c1b1, c1w2, c1b2,
                 g1_lin, g1_as, g1_ad, g1_le, g1_ae, g1_b,
                 g2_lin, g2_as, g2_ad, g2_le, g2_ae, g2_b,
                 mlp_w1, mlp_b1, mlp_w2, mlp_b2,
                 d1w, d1b, d2w, d2b, d3w, d3b,
                 c2w1, c2b1, c2w2, c2b2, c2l1w, c2l1b, c2l2w, c2l2b,
                 edge_index):
    f = np.float32
    x_feat = np.asarray(x_feat, f)
    x_feat_tmp = np.asarray(x_feat_tmp, f)
    edge_attr = np.asarray(edge_attr, f)

    def fill(shape, off, blocks):
        arr = np.zeros(shape, dtype=f)
        for name, a in blocks.items():
            a = np.asarray(a, dtype=f)
            arr[0:a.shape[0], off[name]:off[name] + a.shape[1]] = a
        return arr

    # ---- tA ----
    tpad = np.zeros((8, 2, 12), dtype=f)
    for i in range(8):
        r = 1 if i % 2 == 0 else 5
        for b in range(2):
            tpad[i, b, 1:11] = x_feat_tmp[r, b * 4 + i // 2]
    T3 = np.zeros((24, 20), dtype=f)
    for k in range(3):
        for c in range(8):
            T3[k * 8 + c] = tpad[c, :, k:k + 10].reshape(20)
    W1s = np.asarray(c1w1, f).transpose(2, 1, 0).reshape(24, 8)

    mask16 = np.zeros((16, 8), dtype=f)
    for c in range(2):
        for n in range(8):
            mask16[c * 8 + n, n] = 1.0
    f16 = np.array([(j % 2) * 8 + j // 2 for j in range(16)])
    perm16 = np.array([(f % 2) * 8 + f // 2 for f in f16])
    mask16 = mask16[perm16]

    M24x = np.zeros((16, 24), dtype=f)
    M24x[0:16, 0:16] = np.eye(16, dtype=f)
    M24x[0:16, 16:24] = 1.0 / 16.0

    ve2 = np.asarray(g2_le, f) @ np.asarray(g2_ae, f).reshape(64)  # [64]
    wv2 = (np.asarray(mlp_w2, f) @ ve2).reshape(64, 1)
    e17c = float(np.asarray(mlp_b2, f) @ ve2)

    ipack = np.zeros((24, 50), dtype=np.int32)
    blk = np.zeros((8, 2, 24), dtype=np.int32)
    blk[:, :, 0:16] = np.asarray(edge_index, np.int32)[None, :, :]
    blk[:, :, 16:24] = np.arange(8, dtype=np.int32)[None, None, :]
    ipack[0:8, 0:48] = blk.reshape(8, 48)
    ipack[0:16, 48] = np.asarray(edge_index, np.int32)[1]
    ipack[16:24, 48] = np.arange(8, dtype=np.int32)

    tAm = fill((33, _WA), _oA, {
        "W1s": W1s, "TPAD3": T3,
        "w2T": np.asarray(c1w2, f).transpose(1, 2, 0).reshape(10, 3),
        "mask16": mask16, "M24x": M24x,
        "iota_row24": np.broadcast_to(np.arange(8, dtype=f), (24, 8)),
        "iota8": np.arange(8, dtype=f).reshape(8, 1),
        "ipack": ipack.view(np.float32),
        "c1b1": np.asarray(c1b1, f).reshape(8, 1),
        "c1b2": np.asarray(c1b2, f).reshape(1, 1),
        "c1b2x16": np.broadcast_to(np.asarray(c1b2, f).reshape(1, 1),
                                   (16, 1)),
        "ident16f": np.eye(16, dtype=f),
        "ident8b": _bfpack(np.eye(8, dtype=f)),
    })
    tAm[0, _oA["e17c"]] = e17c

    # ---- tB ----
    xfT = np.zeros((512, 8), dtype=f)
    xfT[0:510] = x_feat.T
    XT4 = xfT.reshape(4, 128, 8).transpose(1, 0, 2).reshape(128, 32)
    G1 = np.asarray(g1_lin, f)                       # [512, 256]
    Was = np.zeros((512, 4), dtype=f)
    Wad = np.zeros((512, 4), dtype=f)
    a_s = np.asarray(g1_as, f)
    a_d = np.asarray(g1_ad, f)
    for h in range(4):
        Was[:, h] = G1[:, h * 64:(h + 1) * 64] @ a_s[h]
        Wad[:, h] = G1[:, h * 64:(h + 1) * 64] @ a_d[h]
    G1f8 = np.concatenate([Was, Wad], 1)             # [512, 8]
    Wae = (np.asarray(g1_le, f).reshape(128, 4, 64) *
           np.asarray(g1_ae, f)[None]).sum(-1)       # [128, 4]
    ea_mean = edge_attr.mean(0)
    ea24 = np.concatenate([edge_attr, np.broadcast_to(ea_mean, (8, 128))], 0)

    tBm = fill((128, _WB), _oB, {
        "XTb": _bfpack(XT4),
        "XTf": XT4,
        "G1b": _bfpack(
            G1.reshape(4, 128, 256).transpose(1, 0, 2).reshape(128, 1024)),
        "G1f": G1f8.reshape(4, 128, 8).transpose(1, 0, 2).reshape(128, 32),
        "W16t": _bfpack(np.repeat(G1[510:512], 8, axis=0)[perm16]),
        "W16as": np.repeat(G1f8[510:512], 8, axis=0)[perm16],
        "eaT24": ea24.T,
        "Wae": Wae,
        "eaTb": _bfpack(edge_attr.T),
        "mlpw1b": _bfpack(np.asarray(mlp_w1, f)),
        "mlpb1": np.asarray(mlp_b1, f).reshape(64, 1),
        "wv2": wv2,
    })

    # ---- tC ----
    G2 = np.asarray(g2_lin, f)                        # [256, 64]
    was2 = (G2 @ np.asarray(g2_as, f).reshape(64)).reshape(256, 1)
    wad2 = (G2 @ np.asarray(g2_ad, f).reshape(64)).reshape(256, 1)
    G2e = np.concatenate([G2, was2, wad2], 1)         # [256, 66]
    G2e4 = G2e.reshape(2, 128, 66).transpose(1, 0, 2).reshape(128, 132)

    mlpw2e = np.zeros((65, 64), dtype=f)
    mlpw2e[0:64] = np.asarray(mlp_w2, f) * 0.125
    mlpw2e[64] = np.asarray(mlp_b2, f)

    D1 = np.concatenate([np.asarray(d1w, f).reshape(64, 640),
                         np.asarray(d2w, f).reshape(64, 640)], 0)
    b4 = np.zeros((4, 640), dtype=f)
    b4[0] = np.repeat(np.asarray(d1b, f), 10)
    b4[1] = x_feat_tmp.reshape(640)
    b4[2] = np.repeat(np.asarray(d2b, f), 10)
    b4[3] = np.repeat(np.asarray(d3b, f), 10)
    D2 = np.concatenate([np.asarray(d3w, f).reshape(64, 640), b4], 0)  # [68,640]

    # CNN2 fold: Wf [128,10] over (c2, l2); A[(c1,j), t]; const row.
    Wf = np.asarray(c2l1w, f) @ np.asarray(c2l2w, f)            # [128, 10]
    bfold = np.asarray(c2l1b, f) @ np.asarray(c2l2w, f) + np.asarray(c2l2b, f)
    WfR = Wf.reshape(64, 2, 10)                                 # [c2, l2, t]
    w2c = np.asarray(c2w2, f)                                   # [64, 32, 3]
    Afold = np.zeros((32, 4, 10), dtype=f)                      # [c1, j, t]
    for j in range(4):
        for l2 in range(2):
            k = j - l2
            if 0 <= k <= 2:
                Afold[:, j, :] += np.einsum("co,ct->ot", w2c[:, :, k],
                                            WfR[:, l2, :])
    const = (np.einsum("c,clt->t",
                       np.asarray(c2b2, f), WfR) + bfold +
             np.einsum("c,cjt->t", np.asarray(c2b1, f), Afold))
    A33m = np.zeros((33, 40), dtype=f)
    A33m[0:32] = Afold.reshape(32, 40)
    A33m[32, 0:10] = const
    tCm = fill((128, _WC), _oC, {
        "G2b": _bfpack(G2e4),
        "mlpw2e": mlpw2e,
        "D1b": _bfpack(D1),
        "D2b": _bfpack(D2),
        "c2w1T": _bfpack(np.asarray(c2w1, f).transpose(1, 2, 0).reshape(4, 96)),
        "A33": _bfpack(A33m),
        "g2brow": np.asarray(g2_b, f).reshape(1, 64),
        "g1brow": _bfpack(np.asarray(g1_b, f).reshape(1, 256)),
        "cidx68": np.broadcast_to(np.arange(4, dtype=f), (68, 4)),
        "pcol68": (np.arange(68, dtype=f) - 64.0).reshape(68, 1),
    })
    return tAm, tBm, tCm


def _make_ins(inputs):
    tAm, tBm, tCm = _pack_inputs(**inputs)
    return {"mA": tAm, "mB": tBm, "mC": tCm}


def kernel(**inputs):
    inputs = {k: np.ascontiguousarray(v) for k, v in inputs.items()}
    ins = _make_ins(inputs)
    nc = _get_nc()
    res = run_bass_kernel_spmd(nc, [ins] * 8, core_ids=list(range(8)))
    return np.ascontiguousarray(res.results[0]["out"].T).reshape(8, 8, 10)


# revision 26
# speedup vs baseline: 1.0535x; 1.0193x over previous
"""DTGNN Trainium2 Bass kernel (v2, latency-optimized).

Single-core algorithm (graph tiny: N=8, E=16), replicated across 8 NeuronCores
via SPMD; core 0's output returned. Optimizations vs v1:
  - bf16 matmul inputs for all wide matmuls (4x PE throughput); fp32 kept on
    the attention/softmax (alpha) path.
  - GAT attention reductions (h*a_s).sum folded into the X@W matmul as extra
    host-precomputed columns (W @ a_s); edge-attn term ae = ea @ (We@a_e).
  - CNN_2 tail (conv2+flatten+linear1+linear2) collapsed into 4 accumulating
    [33,10]x[33,64] matmuls with host-folded weights; conv1 bias commutes
    with maxpool and folds into the same constant row.
  - Biases folded as ones-row contraction tricks (no separate DVE adds).
  - Edge-MLP mean-pool (ef/er) computed as sums of the hidden layer pushed
    through (w2/8 | b2) on the PE.
  - 3 input DMAs (was 6), packed mixed-dtype via bf16-pair bitcasting.
"""
import numpy as np
import ml_dtypes
from contextlib import ExitStack

import concourse.bacc as bacc
import concourse.bass as bass
import concourse.tile as tile
import concourse.mybir as mybir
from concourse.bass_utils import run_bass_kernel_spmd

F32 = mybir.dt.float32
BF16 = mybir.dt.bfloat16
I32 = mybir.dt.int32
ALU = mybir.AluOpType
ACT = mybir.ActivationFunctionType
AXL = mybir.AxisListType


def _mkoff(lst):
    d, o = {}, 0
    for name, w in lst:
        d[name] = o
        o += w
    d["_W"] = o
    return d


# tA [33, *] f32 — constants + CNN1 + one-hot bits (lands first)
_LA = [("W1s", 8), ("TPAD3", 20), ("w2T", 3), ("mask16", 8), ("M24x", 24),
       ("iota_row24", 8), ("iota8", 1), ("ipack", 50), ("c1b1", 1),
       ("c1b2", 1), ("e17c", 1), ("ident16f", 16), ("ident8b", 4),
       ("c1b2x16", 1)]
# tB [128, *] f32 — GAT1 h weights + alpha1 + MLP (lands second)
_LB = [("XTb", 16), ("XTf", 32), ("G1b", 512), ("G1f", 32), ("W16t", 128),
       ("W16as", 8), ("eaT24", 24), ("Wae", 4), ("eaTb", 8), ("mlpw1b", 32),
       ("mlpb1", 1), ("wv2", 1)]
# tC [128, *] f32 — GAT2 + deconv + CNN2 weights (lands third)
_LC = [("G2b", 66), ("mlpw2e", 64), ("D1b", 320), ("D2b", 320),
       ("c2w1T", 48), ("A33", 20), ("g1brow", 128), ("g2brow", 64),
       ("cidx68", 4), ("pcol68", 1)]

_oA, _oB, _oC = _mkoff(_LA), _mkoff(_LB), _mkoff(_LC)
_WA = ((_oA["_W"] + 127) // 128) * 128
_WB = _oB["_W"]
_WC = _oC["_W"]


def _build_nc(stage=99):
    nc = bacc.Bacc("TRN2", target_bir_lowering=False)

    mA = nc.dram_tensor("mA", [33, _WA], F32, kind="ExternalInput")
    mB = nc.dram_tensor("mB", [128, _WB], F32, kind="ExternalInput")
    mC = nc.dram_tensor("mC", [128, _WC], F32, kind="ExternalInput")
    out = nc.dram_tensor("out", [10, 64], F32, kind="ExternalOutput")

    with tile.TileContext(nc) as tc, ExitStack() as ctx:
        sb = ctx.enter_context(tc.tile_pool(name="sb", bufs=1))
        ps = ctx.enter_context(tc.tile_pool(name="ps", bufs=5, space="PSUM"))
        pst = ctx.enter_context(tc.tile_pool(name="pst", bufs=3, space="PSUM"))
        ctx.enter_context(nc.allow_low_precision(reason="bf16 kernel"))

        def _go():
            # --------------------------------------------- input DMAs
            tA = sb.tile([33, _WA], F32)
            nc.sync.dma_start(tA[:], mA[:])
            tB = sb.tile([128, _WB], F32)
            nc.sync.dma_start(tB[:], mB[:])
            tC = sb.tile([128, _WC], F32)
            nc.sync.dma_start(tC[:], mC[:])

            def A(name, w, rows, r0=0):
                return tA[r0:r0 + rows, _oA[name]:_oA[name] + w]

            def Bv(name, w, rows, r0=0):
                return tB[r0:r0 + rows, _oB[name]:_oB[name] + w]

            def C(name, w, rows, r0=0):
                return tC[r0:r0 + rows, _oC[name]:_oC[name] + w]

            W1s = A("W1s", 8, 24)
            TPAD3 = A("TPAD3", 20, 24).rearrange("p (b t) -> p b t", b=2)
            w2T = A("w2T", 3, 10)
            mask16 = A("mask16", 8, 16)
            M24x = A("M24x", 24, 16)
            iota_row24 = A("iota_row24", 8, 24)
            iota8 = A("iota8", 1, 8)
            c1b1 = A("c1b1", 1, 8)
            c1b2x16 = A("c1b2x16", 1, 16)
            e17c = A("e17c", 1, 1)
            ident16f = A("ident16f", 16, 16)
            ident8b = A("ident8b", 4, 8).bitcast(BF16)

            XTb = Bv("XTb", 16, 128).bitcast(BF16).rearrange("p (j n) -> p j n", j=4)
            XTf = Bv("XTf", 32, 128).rearrange("p (j n) -> p j n", j=4)
            G1b = Bv("G1b", 512, 128).bitcast(BF16).rearrange("p (j n) -> p j n", j=4)
            G1f = Bv("G1f", 32, 128).rearrange("p (j n) -> p j n", j=4)
            W16t = Bv("W16t", 128, 16).bitcast(BF16)
            W16as = Bv("W16as", 8, 16)
            eaT24 = Bv("eaT24", 24, 128)
            Wae = Bv("Wae", 4, 128)
            eaTb = Bv("eaTb", 8, 128).bitcast(BF16)
            mlpw1b = Bv("mlpw1b", 32, 128).bitcast(BF16)
            mlpb1 = Bv("mlpb1", 1, 64)
            wv2 = Bv("wv2", 1, 64)

            G2b = C("G2b", 66, 128).bitcast(BF16).rearrange("p (j n) -> p j n", j=2)
            mlpw2e = C("mlpw2e", 64, 65)
            D1b = C("D1b", 320, 128).bitcast(BF16)
            D2b = C("D2b", 320, 68).bitcast(BF16)
            c2w1T = C("c2w1T", 48, 4).bitcast(BF16).rearrange("p (k n) -> p k n", k=3)
            A33 = C("A33", 20, 33).bitcast(BF16).rearrange("p (j n) -> p j n", j=4)
            g1brow = C("g1brow", 128, 1).bitcast(BF16)
            cidx68 = C("cidx68", 4, 68)
            pcol68 = C("pcol68", 1, 68)
            g2brow = C("g2brow", 64, 1)

            # --------------------------------------------- early memsets
            zp = sb.tile([10, 24], F32)
            nc.vector.memset(zp[:], 0.0)
            warm = sb.tile([1, 1], F32)
            nc.vector.memset(warm[:], 0.0)
            warm2 = sb.tile([1, 1], F32)
            nc.scalar.activation(warm2[:], warm[:], ACT.Exp)
            sel = sb.tile([128, 4], BF16)
            nc.vector.memset(sel[:], 0.0)
            selL = sb.tile([64, 4], BF16)
            nc.vector.memset(selL[:], 0.0)
            sel2 = sb.tile([68, 4], BF16)
            Pdst24b = sb.tile([24, 8], BF16)
            y1c33 = sb.tile([33, 64, 4], BF16)
            nc.vector.memset(y1c33[32:33, :, :], 1.0)
            s2 = sb.tile([65, 2], F32)
            nc.vector.memset(s2[64:65, :], 1.0)
            ones1x8 = sb.tile([1, 8], F32)
            nc.vector.memset(ones1x8[:], 1.0)
            ones8bb = sb.tile([1, 8], BF16)
            nc.vector.memset(ones8bb[:], 1.0)
            ones24f = sb.tile([1, 24], F32)
            nc.vector.memset(ones24f[:], 1.0)
            ones8b = sb.tile([8, 1], F32)
            nc.vector.memset(ones8b[:], 0.125)

            # --------------------------------------------- one-hot matrices
            ti = A("ipack", 50, 24).bitcast(I32)
            tif = sb.tile([24, 50], F32)
            nc.vector.tensor_copy(tif[:], ti)
            idx_f = tif[0:8, 0:48].rearrange("p (c e) -> p c e", c=2)
            dcol_f = tif[:, 48:49]

            PsrcTf = sb.tile([8, 24], F32)
            nc.vector.tensor_scalar(PsrcTf[:], idx_f[:, 0, :], iota8, None,
                                    ALU.is_equal)
            PsrcTb = sb.tile([8, 24], BF16)
            nc.vector.tensor_scalar(PsrcTb[:], idx_f[:, 0, :], iota8, None,
                                    ALU.is_equal)
            PdstTf = sb.tile([8, 24], F32)
            nc.vector.tensor_scalar(PdstTf[:], idx_f[:, 1, :], iota8, None,
                                    ALU.is_equal)
            Pdst24f = sb.tile([24, 8], F32)
            nc.vector.tensor_scalar(Pdst24f[:], iota_row24, dcol_f, None,
                                    ALU.is_equal)
            nc.vector.tensor_scalar(Pdst24b[:], iota_row24, dcol_f, None,
                                    ALU.is_equal)

            # --------------------------------------------- CNN_1
            ps_y1 = ps.tile([8, 2, 10], F32, tag="ps")
            nc.tensor.matmul(ps_y1[:], W1s, TPAD3, start=True, stop=True)
            y1 = sb.tile([8, 2, 10], F32)
            nc.vector.tensor_scalar(y1[:], ps_y1[:], c1b1, 0.0, ALU.add, ALU.max)

            ps_za = ps.tile([10, 8], F32, tag="ps")
            nc.tensor.transpose(ps_za[:], y1[:, 0, :], ident16f[0:8, 0:8])
            ps_zb = ps.tile([10, 8], F32, tag="ps")
            nc.tensor.transpose(ps_zb[:], y1[:, 1, :], ident16f[0:8, 0:8])
            zpv = zp[:].rearrange("p (t v) -> p t v", v=2)
            nc.vector.tensor_copy(zpv[:, 1:9, 0], ps_za[:])
            nc.scalar.copy(zpv[:, 1:9, 1], ps_zb[:])

            ps_y2T = ps.tile([16, 1], F32, tag="ps")
            for k in range(3):
                nc.tensor.matmul(ps_y2T[:], zp[:, 2 * k:2 * k + 16],
                                 w2T[:, k:k + 1],
                                 start=(k == 0), stop=(k == 2))
            xv16 = sb.tile([16, 1], F32)
            nc.vector.tensor_scalar(xv16[:], ps_y2T[:], c1b2x16, 0.0,
                                    ALU.add, ALU.max)
            x16f = sb.tile([16, 8], F32)
            nc.vector.tensor_tensor(x16f[:], xv16[:].broadcast_to([16, 8]),
                                    mask16, ALU.mult)
            x16b = sb.tile([16, 8], BF16)
            nc.vector.tensor_tensor(x16b[:], xv16[:].broadcast_to([16, 8]),
                                    mask16, ALU.mult)
            if stage == 1:
                o10 = sb.tile([10, 64], F32)
                nc.vector.memset(o10[:], 0.0)
                nc.vector.tensor_copy(o10[0:8, 0:20],
                                      y1[:].rearrange("p b t -> p (b t)"))
                nc.sync.dma_start(out[:], o10[:])
                return

            # --------------------------------------------- GAT 1
            ps_m1 = ps.tile([64, 16], F32, tag="ps")
            nc.tensor.matmul(ps_m1[:], mlpw1b, eaTb, start=True, stop=True)
            r1T = sb.tile([64, 16], F32)
            nc.vector.tensor_scalar(r1T[:], ps_m1[:], mlpb1, 0.0, ALU.add,
                                    ALU.max)
            r1v = r1T[:].rearrange("p (e two) -> p two e", two=2)
            nc.vector.tensor_reduce(s2[0:64, 0:1], r1v[:, 0, :], axis=AXL.X,
                                    op=ALU.add)
            nc.vector.tensor_reduce(s2[0:64, 1:2], r1v[:, 1, :], axis=AXL.X,
                                    op=ALU.add)

            ps_hf = ps.tile([8, 8], F32, tag="ps")
            for j in range(4):
                nc.tensor.matmul(ps_hf[:], XTf[:, j, :], G1f[:, j, :],
                                 start=(j == 0), stop=False)
            nc.tensor.matmul(ps_hf[:], x16f[:], W16as, start=False, stop=True)
            ps_h = ps.tile([8, 256], F32, tag="ps")
            for j in range(4):
                nc.tensor.matmul(ps_h[:], XTb[:, j, :], G1b[:, j, :],
                                 start=(j == 0), stop=False)
            nc.tensor.matmul(ps_h[:], x16b[:], W16t, start=False, stop=True)

            asad = sb.tile([8, 8], F32)
            nc.vector.tensor_copy(asad[:], ps_hf[:])
            h_sb = sb.tile([8, 256], BF16)
            nc.scalar.copy(h_sb[:], ps_h[:])

            if stage == 21:
                o10 = sb.tile([10, 64], F32)
                nc.vector.memset(o10[:], 0.0)
                nc.vector.tensor_copy(o10[0:8, 0:56], ps_h[:, 0:56])
                nc.vector.tensor_copy(o10[0:8, 56:64], asad[:])
                nc.sync.dma_start(out[:], o10[:])
                return

            ps_al = ps.tile([24, 4], F32, tag="ps")
            nc.tensor.matmul(ps_al[:], eaT24, Wae, start=True, stop=False)
            nc.tensor.matmul(ps_al[:], PsrcTf[:], asad[:, 0:4], start=False,
                             stop=False)
            nc.tensor.matmul(ps_al[:], PdstTf[:], asad[:, 4:8], start=False,
                             stop=True)
            al1 = sb.tile([24, 4], F32)
            nc.vector.tensor_copy(al1[:], ps_al[:])
            ps_sg = ps.tile([24, 256], F32, tag="ps")
            nc.tensor.matmul(ps_sg[:], PsrcTb[:], h_sb[:], start=True, stop=True)
            lr1 = sb.tile([24, 4], F32)
            nc.vector.scalar_tensor_tensor(lr1[:], al1[:], 0.2, al1[:],
                                           ALU.mult, ALU.max)
            ex24 = sb.tile([24, 4], F32)
            nc.scalar.activation(ex24[:], lr1[:], ACT.Exp)
            ps_den = ps.tile([8, 4], F32, tag="ps")
            nc.tensor.matmul(ps_den[:], Pdst24f[:], ex24[:], start=True, stop=True)
            rden = sb.tile([8, 4], F32)
            nc.vector.reciprocal(rden[:], ps_den[:])
            ps_rde = ps.tile([24, 4], F32, tag="ps")
            nc.tensor.matmul(ps_rde[:], PdstTf[:], rden[:], start=True, stop=True)
            wexp = sb.tile([24, 4], F32)
            nc.vector.tensor_tensor(wexp[:], ex24[:], ps_rde[:], ALU.mult)

            if stage == 22:
                o10 = sb.tile([10, 64], F32)
                nc.vector.memset(o10[:], 0.0)
                nc.vector.tensor_copy(o10[0:8, 0:4], rden[:])
                nc.vector.tensor_copy(o10[0:8, 8:12], ps_den[:])
                nc.sync.dma_start(out[:], o10[:])
                return
            wh24 = sb.tile([24, 256], BF16)
            nc.vector.tensor_tensor(
                wh24[:].rearrange("p (h c) -> p h c", h=4),
                ps_sg[:].rearrange("p (h c) -> p h c", h=4),
                wexp[:].broadcast_to([24, 4, 64]), ALU.mult)
            ps_x1 = ps.tile([8, 256], F32, tag="ps")
            nc.tensor.matmul(ps_x1[:], ones8bb[:], g1brow, start=True, stop=False)
            nc.tensor.matmul(ps_x1[:], Pdst24b[:], wh24[:], start=False, stop=True)
            x1 = sb.tile([8, 256], BF16)
            nc.vector.tensor_scalar(x1[:], ps_x1[:], 0.0, None, ALU.max)
            if stage == 2:
                o10 = sb.tile([10, 64], F32)
                nc.vector.memset(o10[:], 0.0)
                nc.vector.tensor_copy(o10[0:8, 0:64], x1[:, 0:64])
                nc.sync.dma_start(out[:], o10[:])
                return

            nc.vector.tensor_scalar(sel2[:], cidx68, pcol68, None, ALU.is_equal)
            # --------------------------------------------- edge MLP (early)
            # ef into partitions 64:128 (for sel), er into 0:64 (for sel2)
            ps_ee = ps.tile([128, 2], F32, tag="ps")
            nc.tensor.matmul(ps_ee[64:128, 0:1], mlpw2e, s2[:, 0:1], start=True,
                             stop=True)
            nc.tensor.matmul(ps_ee[0:64, 1:2], mlpw2e, s2[:, 1:2], start=True,
                             stop=True)
            nc.vector.tensor_copy(sel[64:128, 2:3], ps_ee[64:128, 0:1])
            nc.vector.tensor_copy(sel2[0:64, 3:4], ps_ee[0:64, 1:2])

            ps_e16 = ps.tile([16, 1], F32, tag="ps")
            nc.tensor.matmul(ps_e16[:], r1T[:], wv2, start=True, stop=True)
            e16sb = sb.tile([16, 1], F32)
            nc.vector.tensor_copy(e16sb[:], ps_e16[:])
            b8sb = sb.tile([8, 64], F32)
            ps_b8 = ps.tile([8, 64], F32, tag="ps")
            nc.tensor.matmul(ps_b8[:], ones1x8[:], g2brow, start=True, stop=True)
            nc.vector.tensor_copy(b8sb[:], ps_b8[:])

            # --------------------------------------------- GAT 2
            ps_tr1 = ps.tile([128, 8], BF16, tag="ps")
            nc.tensor.transpose(ps_tr1[:], x1[:, 0:128], ident8b)
            ps_tr2 = ps.tile([128, 8], BF16, tag="ps")
            nc.tensor.transpose(ps_tr2[:], x1[:, 128:256], ident8b)
            x1T = sb.tile([128, 2, 8], BF16)
            nc.vector.tensor_copy(x1T[:, 0, :], ps_tr1[:])
            nc.vector.tensor_copy(x1T[:, 1, :], ps_tr2[:])

            ps_h2 = ps.tile([8, 66], F32, tag="ps")
            for j in range(2):
                nc.tensor.matmul(ps_h2[:], x1T[:, j, :], G2b[:, j, :],
                                 start=(j == 0), stop=(j == 1))
            hs2 = sb.tile([8, 66], F32)
            nc.vector.tensor_copy(hs2[:], ps_h2[:])

            ps_al2 = ps.tile([24, 1], F32, tag="ps")
            nc.tensor.matmul(ps_al2[:], ones24f[:], e17c, start=True, stop=False)
            nc.tensor.matmul(ps_al2[:], M24x, e16sb[:], start=False, stop=False)
            nc.tensor.matmul(ps_al2[:], PsrcTf[:], hs2[:, 64:65], start=False,
                             stop=False)
            nc.tensor.matmul(ps_al2[:], PdstTf[:], hs2[:, 65:66], start=False,
                             stop=True)
            al2 = sb.tile([24, 1], F32)
            nc.vector.tensor_copy(al2[:], ps_al2[:])
            lr2 = sb.tile([24, 1], F32)
            nc.vector.scalar_tensor_tensor(lr2[:], al2[:], 0.2, al2[:],
                                           ALU.mult, ALU.max)
            ex2 = sb.tile([24, 1], F32)
            nc.scalar.activation(ex2[:], lr2[:], ACT.Exp)
            ps_sg2 = ps.tile([24, 64], F32, tag="ps")
            nc.tensor.matmul(ps_sg2[:], PsrcTf[:], hs2[:, 0:64], start=True,
                             stop=True)
            ps_cTb = pst.tile([4, 320], F32, tag="pst")
            nc.tensor.matmul(ps_cTb[:], sel[:], D1b[:, 320:640], start=True,
                             stop=False)
            nc.tensor.matmul(ps_cTb[:], sel2[:], D2b[:, 320:640], start=False,
                             stop=False)
            ps_cTa = pst.tile([4, 320], F32, tag="pst")
            nc.tensor.matmul(ps_cTa[:], sel[:], D1b[:, 0:320], start=True,
                             stop=False)
            nc.tensor.matmul(ps_cTa[:], sel2[:], D2b[:, 0:320], start=False,
                             stop=False)
            ps_den2 = ps.tile([8, 1], F32, tag="ps")
            nc.tensor.matmul(ps_den2[:], Pdst24f[:], ex2[:], start=True, stop=True)
            rden2 = sb.tile([8, 1], F32)
            nc.vector.reciprocal(rden2[:], ps_den2[:])
            wh2 = sb.tile([24, 64], BF16)
            nc.vector.tensor_scalar(wh2[:], ps_sg2[:], ex2[:], None, ALU.mult)
            ps_x2u = ps.tile([8, 64], F32, tag="ps")
            nc.tensor.matmul(ps_x2u[:], Pdst24b[:], wh2[:], start=True,
                             stop=True)
            x2t = sb.tile([8, 64], F32)
            nc.vector.scalar_tensor_tensor(x2t[:], ps_x2u[:], rden2[:], b8sb[:],
                                           ALU.mult, ALU.add)
            x2 = sb.tile([8, 64], F32)
            nc.vector.tensor_scalar(x2[:], x2t[:], 0.0, None, ALU.max)
            if stage == 3:
                o10 = sb.tile([10, 64], F32)
                nc.vector.memset(o10[:], 0.0)
                nc.vector.tensor_copy(o10[0:8, 0:64], x2[:])
                nc.sync.dma_start(out[:], o10[:])
                return

            # --------------------------------------------- deconv selector
            ps_xm = ps.tile([64, 1], F32, tag="ps")
            nc.tensor.matmul(ps_xm[:], x2[:], ones8b[:], start=True, stop=True)
            nc.vector.tensor_copy(selL[0:64, 0:1], ps_xm[:])

            nc.tensor.matmul(ps_cTb[:], selL[:], D1b[0:64, 320:640],
                             start=False, stop=True)
            nc.tensor.matmul(ps_cTa[:], selL[:], D1b[0:64, 0:320],
                             start=False, stop=True)
            cT = sb.tile([4, 640], BF16)
            nc.scalar.copy(cT[:, 320:640], ps_cTb[:])
            nc.vector.tensor_copy(cT[:, 0:320], ps_cTa[:])
            if stage == 4:
                o10 = sb.tile([10, 64], F32)
                nc.vector.memset(o10[:], 0.0)
                nc.vector.tensor_copy(o10[0:4, 0:40], cT[:, 0:40])
                nc.sync.dma_start(out[:], o10[:])
                return

            # --------------------------------------------- CNN_2
            cTv = cT[:].rearrange("p (b l) -> p b l", b=64)
            ps_c1b = pst.tile([32, 32, 8], F32, tag="pst")
            for k in range(3):
                nc.tensor.matmul(ps_c1b[:], c2w1T[:, k, :],
                                 cTv[:, 32:64, k:k + 8],
                                 start=(k == 0), stop=(k == 2))
            ps_c1a = pst.tile([32, 32, 8], F32, tag="pst")
            for k in range(3):
                nc.tensor.matmul(ps_c1a[:], c2w1T[:, k, :],
                                 cTv[:, 0:32, k:k + 8],
                                 start=(k == 0), stop=(k == 2))
            nc.vector.tensor_reduce(
                y1c33[0:32, 32:64, :],
                ps_c1b[:].rearrange("p b (l two) -> p b l two", two=2),
                axis=AXL.X, op=ALU.max)
            nc.vector.tensor_reduce(
                y1c33[0:32, 0:32, :],
                ps_c1a[:].rearrange("p b (l two) -> p b l two", two=2),
                axis=AXL.X, op=ALU.max)

            o10 = sb.tile([10, 64], F32)
            ps_outB = ps.tile([10, 32], F32, tag="ps")
            for j in range(4):
                nc.tensor.matmul(ps_outB[:], A33[:, j, :], y1c33[:, 32:64, j],
                                 start=(j == 0), stop=(j == 3))
            nc.vector.tensor_scalar(o10[:, 32:64], ps_outB[:], 0.0, None,
                                    ALU.max)
            ps_outA = ps.tile([10, 32], F32, tag="ps")
            for j in range(4):
                nc.tensor.matmul(ps_outA[:], A33[:, j, :], y1c33[:, 0:32, j],
                                 start=(j == 0), stop=(j == 3))
            nc.vector.tensor_scalar(o10[:, 0:32], ps_outA[:], 0.0, None,
                                    ALU.max)
            nc.sync.dma_start(out[:], o10[:])

        _go()
    nc.finalize()
    return nc


_NC = None


def _get_nc():
    global _NC
    if _NC is None:
        _NC = _build_nc()
    return _NC


def _bfpack(a):
    """[r, c] float array -> [r, c/2] f32 whose bits hold bf16 pairs."""
    a = np.ascontiguousarray(np.asarray(a, dtype=np.float32))
    r, c = a.shape
    assert c % 2 == 0, c
    u = a.astype(ml_dtypes.bfloat16).view(np.uint16).reshape(r, c // 2, 2)
    packed = u[:, :, 0].astype(np.uint32) | (u[:, :, 1].astype(np.uint32) << 16)
    return packed.view(np.float32)


def _pack_inputs(x_feat, x_feat_tmp, edge_attr, c1w1, c1b1, c1w2, c1b2,
                 g1_lin, g1_as, g1_ad, g1_le, g1_ae, g1_b,
                 g2_lin, g2_as, g2_ad, g2_le, g2_ae, g2_b,
                 mlp_w1, mlp_b1, mlp_w2, mlp_b2,
                 d1w, d1b, d2w, d2b, d3w, d3b,
                 c2w1, c2b1, c2w2, c2b2, c2l1w, c2l1b, c2l2w, c2l2b,
                 edge_index):
    f = np.float32
    x_feat = np.asarray(x_feat, f)
    x_feat_tmp = np.asarray(x_feat_tmp, f)
    edge_attr = np.asarray(edge_attr, f)

    def fill(shape, off, blocks):
        arr = np.zeros(shape, dtype=f)
        for name, a in blocks.items():
            a = np.asarray(a, dtype=f)
            arr[0:a.shape[0], off[name]:off[name] + a.shape[1]] = a
        return arr

    # ---- tA ----
    tpad = np.zeros((8, 2, 12), dtype=f)
    for i in range(8):
        r = 1 if i % 2 == 0 else 5
        for b in range(2):
            tpad[i, b, 1:11] = x_feat_tmp[r, b * 4 + i // 2]
    T3 = np.zeros((24, 20), dtype=f)
    for k in range(3):
        for c in range(8):
            T3[k * 8 + c] = tpad[c, :, k:k + 10].reshape(20)
    W1s = np.asarray(c1w1, f).transpose(2, 1, 0).reshape(24, 8)

    mask16 = np.zeros((16, 8), dtype=f)
    for c in range(2):
        for n in range(8):
            mask16[c * 8 + n, n] = 1.0
    f16 = np.array([(j % 2) * 8 + j // 2 for j in range(16)])
    perm16 = np.array([(f % 2) * 8 + f // 2 for f in f16])
    mask16 = mask16[perm16]

    M24x = np.zeros((16, 24), dtype=f)
    M24x[0:16, 0:16] = np.eye(16, dtype=f)
    M24x[0:16, 16:24] = 1.0 / 16.0

    ve2 = np.asarray(g2_le, f) @ np.asarray(g2_ae, f).reshape(64)  # [64]
    wv2 = (np.asarray(mlp_w2, f) @ ve2).reshape(64, 1)
    e17c = float(np.asarray(mlp_b2, f) @ ve2)

    ipack = np.zeros((24, 50), dtype=np.int32)
    blk = np.zeros((8, 2, 24), dtype=np.int32)
    blk[:, :, 0:16] = np.asarray(edge_index, np.int32)[None, :, :]
    blk[:, :, 16:24] = np.arange(8, dtype=np.int32)[None, None, :]
    ipack[0:8, 0:48] = blk.reshape(8, 48)
    ipack[0:16, 48] = np.asarray(edge_index, np.int32)[1]
    ipack[16:24, 48] = np.arange(8, dtype=np.int32)

    tAm = fill((33, _WA), _oA, {
        "W1s": W1s, "TPAD3": T3,
        "w2T": np.asarray(c1w2, f).transpose(1, 2, 0).reshape(10, 3),
        "mask16": mask16, "M24x": M24x,
        "iota_row24": np.broadcast_to(np.arange(8, dtype=f), (24, 8)),
        "iota8": np.arange(8, dtype=f).reshape(8, 1),
        "ipack": ipack.view(np.float32),
        "c1b1": np.asarray(c1b1, f).reshape(8, 1),
        "c1b2": np.asarray(c1b2, f).reshape(1, 1),
        "c1b2x16": np.broadcast_to(np.asarray(c1b2, f).reshape(1, 1),
                                   (16, 1)),
        "ident16f": np.eye(16, dtype=f),
        "ident8b": _bfpack(np.eye(8, dtype=f)),
    })
    tAm[0, _oA["e17c"]] = e17c

    # ---- tB ----
    xfT = np.zeros((512, 8), dtype=f)
    xfT[0:510] = x_feat.T
    XT4 = xfT.reshape(4, 128, 8).transpose(1, 0, 2).reshape(128, 32)
    G1 = np.asarray(g1_lin, f)                       # [512, 256]
    Was = np.zeros((512, 4), dtype=f)
    Wad = np.zeros((512, 4), dtype=f)
    a_s = np.asarray(g1_as, f)
    a_d = np.asarray(g1_ad, f)
    for h in range(4):
        Was[:, h] = G1[:, h * 64:(h + 1) * 64] @ a_s[h]
        Wad[:, h] = G1[:, h * 64:(h + 1) * 64] @ a_d[h]
    G1f8 = np.concatenate([Was, Wad], 1)             # [512, 8]
    Wae = (np.asarray(g1_le, f).reshape(128, 4, 64) *
           np.asarray(g1_ae, f)[None]).sum(-1)       # [128, 4]
    ea_mean = edge_attr.mean(0)
    ea24 = np.concatenate([edge_attr, np.broadcast_to(ea_mean, (8, 128))], 0)

    tBm = fill((128, _WB), _oB, {
        "XTb": _bfpack(XT4),
        "XTf": XT4,
        "G1b": _bfpack(
            G1.reshape(4, 128, 256).transpose(1, 0, 2).reshape(128, 1024)),
        "G1f": G1f8.reshape(4, 128, 8).transpose(1, 0, 2).reshape(128, 32),
        "W16t": _bfpack(np.repeat(G1[510:512], 8, axis=0)[perm16]),
        "W16as": np.repeat(G1f8[510:512], 8, axis=0)[perm16],
        "eaT24": ea24.T,
        "Wae": Wae,
        "eaTb": _bfpack(edge_attr.T),
        "mlpw1b": _bfpack(np.asarray(mlp_w1, f)),
        "mlpb1": np.asarray(mlp_b1, f).reshape(64, 1),
        "wv2": wv2,
    })

    # ---- tC ----
    G2 = np.asarray(g2_lin, f)                        # [256, 64]
    was2 = (G2 @ np.asarray(g2_as, f).reshape(64)).reshape(256, 1)
    wad2 = (G2 @ np.asarray(g2_ad, f).reshape(64)).reshape(256, 1)
    G2e = np.concatenate([G2, was2, wad2], 1)         # [256, 66]
    G2e4 = G2e.reshape(2, 128, 66).transpose(1, 0, 2).reshape(128, 132)

    mlpw2e = np.zeros((65, 64), dtype=f)
    mlpw2e[0:64] = np.asarray(mlp_w2, f) * 0.125
    mlpw2e[64] = np.asarray(mlp_b2, f)

    D1 = np.concatenate([np.asarray(d1w, f).reshape(64, 640),
                         np.asarray(d2w, f).reshape(64, 640)], 0)
    b4 = np.zeros((4, 640), dtype=f)
    b4[0] = np.repeat(np.asarray(d1b, f), 10)
    b4[1] = x_feat_tmp.reshape(640)
    b4[2] = np.repeat(np.asarray(d2b, f), 10)
    b4[3] = np.repeat(np.asarray(d3b, f), 10)
    D2 = np.concatenate([np.asarray(d3w, f).reshape(64, 640), b4], 0)  # [68,640]

    # CNN2 fold: Wf [128,10] over (c2, l2); A[(c1,j), t]; const row.
    Wf = np.asarray(c2l1w, f) @ np.asarray(c2l2w, f)            # [128, 10]
    bfold = np.asarray(c2l1b, f) @ np.asarray(c2l2w, f) + np.asarray(c2l2b, f)
    WfR = Wf.reshape(64, 2, 10)                                 # [c2, l2, t]
    w2c = np.asarray(c2w2, f)                                   # [64, 32, 3]
    Afold = np.zeros((32, 4, 10), dtype=f)                      # [c1, j, t]
    for j in range(4):
        for l2 in range(2):
            k = j - l2
            if 0 <= k <= 2:
                Afold[:, j, :] += np.einsum("co,ct->ot", w2c[:, :, k],
                                            WfR[:, l2, :])
    const = (np.einsum("c,clt->t",
                       np.asarray(c2b2, f), WfR) + bfold +
             np.einsum("c,cjt->t", np.asarray(c2b1, f), Afold))
    A33m = np.zeros((33, 40), dtype=f)
    A33m[0:32] = Afold.reshape(32, 40)
    A33m[32, 0:10] = const
    tCm = fill((128, _WC), _oC, {
        "G2b": _bfpack(G2e4),
        "mlpw2e": mlpw2e,
        "D1b": _bfpack(D1),
        "D2b": _bfpack(D2),
        "c2w1T": _bfpack(np.asarray(c2w1, f).transpose(1, 2, 0).reshape(4, 96)),
        "A33": _bfpack(A33m),
        "g2brow": np.asarray(g2_b, f).reshape(1, 64),
        "g1brow": _bfpack(np.asarray(g1_b, f).reshape(1, 256)),
        "cidx68": np.broadcast_to(np.arange(4, dtype=f), (68, 4)),
        "pcol68": (np.arange(68, dtype=f) - 64.0).reshape(68, 1),
    })
    return tAm, tBm, tCm


def _make_ins(inputs):
    tAm, tBm, tCm = _pack_inputs(**inputs)
    return {"mA": tAm, "mB": tBm, "mC": tCm}


def kernel(**inputs):
    inputs = {k: np.ascontiguousarray(v) for k, v in inputs.items()}
    ins = _make_ins(inputs)
    nc = _get_nc()
    res = run_bass_kernel_spmd(nc, [ins] * 8, core_ids=list(range(8)))
    return np.ascontiguousarray(res.results[0]["out"].T).reshape(8, 8, 10)
